# revision 1
# baseline (speedup 1.0000x reference)
"""Multi-head attention (B=4, S=2048, D=1024, H=16, DH=64) on 8 Trainium2
NeuronCores.

Sharding: core c handles batch b = c//2 and head-group g = c%2 (8 heads,
i.e. columns 512g:512(g+1) of Wq/Wk/Wv and rows 512g:512(g+1) of Wo).
Each core produces a partial output projection; the host sums the two
partials per batch and adds bo. No collectives.

Device kernel (per core, everything bf16 with fp32 PSUM accumulation):
  A. QT = Wq_g^T @ xq^T   [512, 2048]   (likewise KT), V = xv @ Wv_g
     stored interleaved with a ones column per head ("vext").
  B. Per head h, per key-tile j: scoresT[k, q] = KT_j^T-stationary matmul,
     exp via ScalarE (scale=1/sqrt(DH) folded in, no max subtraction -- the
     scores are bounded), causal/masked tiles handled by host-computed tile
     classification (skip / elementwise-multiply).  PV matmul with
     lhsT = [V | ones] accumulates unnormalized outT plus the softmax
     denominators Z in one pass.
  C. Normalize: xT *= broadcast(1/Z) (indicator-matrix matmul broadcast).
  D. yT = Wo_g-stationary projection of xT, written transposed; host
     re-transposes.
"""

import numpy as np
import ml_dtypes

import concourse.bacc as bacc
import concourse.mybir as mybir
import concourse.tile as tile
from concourse import bass_utils

BF16 = mybir.dt.bfloat16
F32 = mybir.dt.float32
F32R = mybir.dt.float32r
EXP = mybir.ActivationFunctionType.Exp

B, S, D, H, DH = 4, 2048, 1024, 16, 64
P = 128
NT = S // P            # 16 key/query tiles
GROUPS = 2             # head groups (tensor parallel)
HG = H // GROUPS       # 8 heads per core
DG = D // GROUPS       # 512
KD = D // P            # 8 contraction tiles over D
TD = DG // P           # 4 d-tiles per group
HC = DH + 1            # 65: V columns + ones column per head
SCALE = float(DH) ** -0.5
BANK = 512             # fp32 PSUM bank, in elements
MAX_PRELOAD_MASK = 64

_cache = {}
_last_results = None


def _plan_from_mask(mask_bool, has_bias):
    g = mask_bool.reshape(NT, P, NT, P).sum(axis=(1, 3))
    full = g == P * P
    zero = g == 0

    mixed_tiles = []        # ordered list of (i, j)
    mixed_of = {}           # (i, j) -> index into mixed_tiles
    j_info = [None] * NT
    for j in range(NT):
        act = [i for i in range(NT) if not zero[i, j]]
        if not act:
            continue
        i0, i1 = min(act), max(act) + 1
        mixed = []
        for i in range(i0, i1):
            if not full[i, j]:
                if (i, j) not in mixed_of:
                    mixed_of[(i, j)] = len(mixed_tiles)
                    mixed_tiles.append((i, j))
                mixed.append((i, mixed_of[(i, j)]))
        qlo, qhi = i0 * P, i1 * P
        # one slab per 1024-wide q-half; slab PSUM tile base s0a is
        # 512-aligned so the 512-aligned matmul chunks never cross a bank
        # inside the tile.
        slabs = {}
        for half in range(2):
            qb = max(qlo, half * 1024)
            qe = min(qhi, (half + 1) * 1024)
            if qb >= qe:
                continue
            s0a = (qb // BANK) * BANK
            chunks = []
            d = qb
            while d < qe:
                d2 = min((d // BANK + 1) * BANK, qe)
                chunks.append((d, d2))
                d = d2
            slabs[half] = (s0a, qb, qe, chunks)
        j_info[j] = dict(qlo=qlo, qhi=qhi, slabs=slabs, mixed=mixed)

    first_j = {}
    last_j = {}
    for j in range(NT):
        if j_info[j] is None:
            continue
        for (_, _, _, chunks) in j_info[j]["slabs"].values():
            for (c0, _) in chunks:
                bk = c0 // BANK
                first_j.setdefault(bk, j)
                last_j[bk] = j
    # split-schedule legality: q-half-0 attention touches only key-half-0
    # (true for causal), so projections can be computed half-by-half with
    # attention interleaved between them
    split = all(j_info[j] is None or 0 not in j_info[j]["slabs"]
                for j in range(NT // 2, NT))
    return dict(
        j_info=j_info,
        mixed_tiles=mixed_tiles,
        first_j=first_j,
        last_j=last_j,
        has_bias=has_bias,
        split=split,
    )


def _build(plan):
    has_bias = plan["has_bias"]
    j_info = plan["j_info"]
    mixed_tiles = plan["mixed_tiles"]
    nm = max(1, len(mixed_tiles))
    preload = len(mixed_tiles) <= MAX_PRELOAD_MASK

    nc = bacc.Bacc("TRN2", target_bir_lowering=False, debug=False)
    xq_d = nc.dram_tensor("xq", [D, S], BF16, kind="ExternalInput").ap()
    xk_d = nc.dram_tensor("xk", [D, S], BF16, kind="ExternalInput").ap()
    xv_d = nc.dram_tensor("xv", [D, S], BF16, kind="ExternalInput").ap()
    wq_d = nc.dram_tensor("wq", [D, DG], BF16, kind="ExternalInput").ap()
    wk_d = nc.dram_tensor("wk", [D, DG], BF16, kind="ExternalInput").ap()
    wv_d = nc.dram_tensor("wv", [D, DG], BF16, kind="ExternalInput").ap()
    wo_d = nc.dram_tensor("wo", [DG, D], BF16, kind="ExternalInput").ap()
    ind_d = nc.dram_tensor("ind", [HC, 2 * P], F32R, kind="ExternalInput").ap()
    mm_d = nc.dram_tensor("mmask", [nm, P, P], BF16, kind="ExternalInput").ap()
    if has_bias:
        bq_d = nc.dram_tensor("bq", [1, DG], BF16, kind="ExternalInput").ap()
        bk_d = nc.dram_tensor("bk", [1, DG], BF16, kind="ExternalInput").ap()
        bv_d = nc.dram_tensor("bv", [1, DG], BF16, kind="ExternalInput").ap()
    y_d = nc.dram_tensor("yT", [D, S], F32, kind="ExternalOutput").ap()

    xq_t = xq_d.rearrange("(n p) q -> n p q", p=P)
    xk_t = xk_d.rearrange("(n p) q -> n p q", p=P)
    xv_t = xv_d.rearrange("(n p) q -> n p q", p=P)
    wq_t = wq_d.rearrange("(n p) d -> n p d", p=P)
    wk_t = wk_d.rearrange("(n p) d -> n p d", p=P)
    wv_t = wv_d.rearrange("(n p) d -> n p d", p=P)
    wo_t = wo_d.rearrange("(n p) e -> n p e", p=P)
    y_t = y_d.rearrange("(n p) q -> n p q", p=P)

    with tile.TileContext(nc, trace_sim=False) as tc:
        with (
            tc.tile_pool(name="pers", bufs=1) as pers,
            tc.tile_pool(name="xin", bufs=12) as xin,
            tc.tile_pool(name="win", bufs=10) as win,
            tc.tile_pool(name="ptp", bufs=4) as ptp,
            tc.tile_pool(name="tmpp", bufs=1) as tmpp,
            tc.tile_pool(name="outp", bufs=2) as outp,
        ):
            # ---- persistent SBUF tensors -------------------------------
            qt = [pers.tile([P, S], BF16, tag="qt", bufs=TD, name=f"qt{t}")
                  for t in range(TD)]
            kt = [pers.tile([P, S], BF16, tag="kt", bufs=TD, name=f"kt{t}")
                  for t in range(TD)]
            vx = [pers.tile([P, HG * HC], BF16, tag="vx", bufs=NT,
                            name=f"vx{j}") for j in range(NT)]
            xtu = [pers.tile([P, S], BF16, tag="xtu", bufs=TD, name=f"xtu{t}")
                   for t in range(TD)]
            # head-parity selector rows for the 1/Z broadcast matmuls; row 64
            # so the base partition matches the ztmp Z-rows (bass requires
            # equal lhsT/rhs base partitions)
            ind_s = pers.tile([HC, 2 * P], F32R, tag="ind", bufs=1,
                              name="ind_s")
            wo_s = [pers.tile([P, D], BF16, tag="wo", bufs=TD, name=f"wo{t}")
                    for t in range(TD)]

            mtile = {}
            if preload:
                for idx, (i, j) in enumerate(mixed_tiles):
                    mtile[(i, j)] = pers.tile([P, P], BF16, tag="mt", bufs=nm,
                                              name=f"mt{idx}")

            if has_bias:
                ones = pers.tile([1, BANK], BF16, tag="ones", bufs=1,
                                 name="ones")
                nc.vector.memset(ones[:], 1.0)
                bias_s = {}
                for nm_, d_ in (("bq", bq_d), ("bk", bk_d), ("bv", bv_d)):
                    bs = pers.tile([1, DG], BF16, tag="bias", bufs=3,
                                   name=f"{nm_}_s")
                    nc.sync.dma_start(bs[:], d_)
                    bias_s[nm_] = bs

            # ---- Phases A+B share one PSUM pool ------------------------
            #   tag "pp" (2x2 banks): projection psums (A), pout tiles (B)
            #   tag "sc" (2x2 banks): V-proj psums, score slabs, zb tiles
            # Sharing tags across phases keeps the slots flowing with no
            # pool-boundary barrier, so V-proj overlaps early attention.
            with tc.tile_pool(name="psAB", bufs=2, space="PSUM") as ps:
                split = plan["split"]
                xin_b = 26 if split else 12
                win_b = 24 if split else 16
                ztmp_b = 6 if split else 8

                def load_w8(wd, label):
                    ts_ = []
                    for i in range(KD):
                        wt_ = win.tile([P, DG], BF16, tag="w", bufs=win_b,
                                       name=f"w{label}{i}")
                        nc.sync.dma_start(wt_[:], wd[i])
                        ts_.append(wt_)
                    return ts_

                def load_x8(xd, label, half):
                    # half=None: full rows (serial); else one 1024-col half
                    w = S if half is None else 1024
                    off = 0 if half is None else 1024 * half
                    ts_ = []
                    for i in range(KD):
                        xt_ = xin.tile([P, w], BF16, tag="x", bufs=xin_b,
                                       name=f"x{label}{i}")
                        nc.sync.dma_start(xt_[:], xd[i][:, off:off + w])
                        ts_.append(xt_)
                    return ts_

                def proj_qk_t(xs, ws, bias, out_tiles, label, half, xoff, t):
                    # out_tiles[t][:, half cols] = sum_i ws[i][:,t]^T @ xs[i]
                    pp = ps.tile([P, 1024], F32, tag="pp",
                                 name=f"ps_{label}{t}_{half}")
                    for i in range(KD):
                        for cs in range(2):
                            x0 = xoff + cs * BANK
                            nc.tensor.matmul(
                                pp[:, cs * BANK:(cs + 1) * BANK],
                                ws[i][:, t * P:(t + 1) * P],
                                xs[i][:, x0:x0 + BANK],
                                start=(i == 0),
                                stop=(i == KD - 1 and bias is None),
                            )
                    if bias is not None:
                        for cs in range(2):
                            nc.tensor.matmul(
                                pp[:, cs * BANK:(cs + 1) * BANK],
                                bias[0:1, t * P:(t + 1) * P],
                                ones[0:1, :],
                                start=False, stop=(cs == 1),
                            )
                    nc.vector.tensor_copy(
                        out_tiles[t][:, half * 1024:(half + 1) * 1024],
                        pp[:],
                    )

                def proj_qk(xs, ws, bias, out_tiles, label, half, xoff):
                    for t in range(TD):
                        proj_qk_t(xs, ws, bias, out_tiles, label, half,
                                  xoff, t)

                def proj_v(xs, jrange, xoff_base):
                    for j in jrange:
                        lc = j * P - xoff_base
                        psv = ps.tile([P, DG], F32, tag="pp", name=f"ps_v{j}")
                        for i in range(KD):
                            nc.tensor.matmul(
                                psv[:],
                                xs[i][:, lc:lc + P],
                                ws3["v"][i][:],
                                start=(i == 0),
                                stop=(i == KD - 1 and not has_bias),
                            )
                        if has_bias:
                            nc.tensor.matmul(
                                psv[:], ones[0:1, 0:P], bias_s["bv"][0:1, :],
                                start=False, stop=True,
                            )
                        vxv = vx[j][:].rearrange("p (g c) -> p g c", c=HC)
                        nc.vector.memset(vxv[:, :, DH:HC], 1.0)
                        nc.vector.tensor_copy(
                            vxv[:, :, 0:DH],
                            psv[:].rearrange("p (g c) -> p g c", c=DH),
                        )

                def late_loads():
                    # needed only from phase B onward; emitted after the x/w
                    # loads so they queue behind them on the DMA engines
                    nc.sync.dma_start(ind_s[:], ind_d)
                    for t in range(TD):
                        nc.sync.dma_start(wo_s[t][:], wo_t[t])
                    if preload:
                        for idx, (i, j) in enumerate(mixed_tiles):
                            nc.sync.dma_start(mtile[(i, j)][:], mm_d[idx])

                ztmps = {}

                def emit_norm_half(t, half):
                    # normalize xtu[t] q-half by 1/Z of head pair (2t, 2t+1)
                    zb = ps.tile([P, 1024], F32, tag="pp",
                                 name=f"zb{t}_{half}")
                    for hh in range(2):
                        zt_ = ztmps[(2 * t + hh, half)]
                        for cs in range(2):
                            nc.tensor.matmul(
                                zb[:, cs * BANK:(cs + 1) * BANK],
                                ind_s[DH:HC, hh * P:(hh + 1) * P],
                                zt_[DH:HC, cs * BANK:(cs + 1) * BANK],
                                start=(hh == 0), stop=(hh == 1),
                            )
                    nc.vector.tensor_mul(
                        xtu[t][:, half * 1024:(half + 1) * 1024],
                        xtu[t][:, half * 1024:(half + 1) * 1024],
                        zb[:],
                    )

                def emit_head_half(h, half, mid=None):
                    # `mid` = filler work (projection units, deferred norms,
                    # output-projection groups) emitted after the 4th key
                    # tile: mid-head DVE is idle, so the fillers' PSUM slots
                    # release promptly instead of queueing behind the
                    # head-boundary copy burst and starving ScalarE
                    t, r0 = h // 2, DH * (h % 2)
                    h0, h1 = half * 1024, (half + 1) * 1024
                    pout_t = ps.tile([P, 1024], F32, tag="pp",
                                     name=f"pout{h}_{half}")
                    pout = pout_t[0:HC]
                    wrote = False
                    for j in range(NT):
                        info = j_info[j]
                        if info is None or half not in info["slabs"]:
                            continue
                        (s0, qb, s1, chunks) = info["slabs"][half]
                        ps_s = ps.tile([P, 1024], F32, tag="sc",
                                       name=f"sc{h}_{j}_{half}")
                        for (c0, c1) in chunks:
                            nc.tensor.matmul(
                                ps_s[:, c0 - s0:c1 - s0],
                                kt[t][r0:r0 + DH, j * P:(j + 1) * P],
                                qt[t][r0:r0 + DH, c0:c1],
                                start=True, stop=True,
                            )
                        pt = ptp.tile([P, 1024], BF16, tag="pt", bufs=4,
                                      name=f"pt{h}_{j}_{half}")
                        nc.scalar.activation(
                            pt[:, qb - s0:s1 - s0],
                            ps_s[:, qb - s0:s1 - s0], EXP,
                            scale=SCALE,
                        )
                        for (i, idx) in info["mixed"]:
                            ic = i * P
                            if not (qb <= ic < s1):
                                continue
                            if preload:
                                mt = mtile[(i, j)]
                            else:
                                mt = ptp.tile([P, P], BF16, tag="mts",
                                              bufs=4, name=f"mts{h}_{j}_{i}")
                                nc.sync.dma_start(mt[:], mm_d[idx])
                            nc.vector.tensor_mul(
                                pt[:, ic - s0:ic - s0 + P],
                                pt[:, ic - s0:ic - s0 + P],
                                mt[:],
                            )
                        for (c0, c1) in chunks:
                            bk_ = c0 // BANK
                            nc.tensor.matmul(
                                pout[:, c0 - h0:c1 - h0],
                                vx[j][:, h * HC:(h + 1) * HC],
                                pt[:, c0 - s0:c1 - s0],
                                start=(j == plan["first_j"][bk_]),
                                stop=(j == plan["last_j"][bk_]),
                            )
                        wrote = True
                    if mid:
                        for fn_, args_ in mid:
                            fn_(*args_)
                    if not wrote:
                        return
                    # copy unnormalized head output + denominators out
                    if r0 == 0:
                        nc.vector.tensor_copy(xtu[t][0:DH, h0:h1],
                                              pout[0:DH, :])
                    else:
                        xtmp = tmpp.tile([DH, 1024], BF16, tag="xtmp",
                                         bufs=2, name=f"xtmp{h}_{half}")
                        nc.vector.tensor_copy(xtmp[:], pout[0:DH, :])
                        nc.sync.dma_start(xtu[t][DH:P, h0:h1], xtmp[:])
                    ztmp = tmpp.tile([HC, 1024], F32R, tag="ztmp",
                                     bufs=ztmp_b, name=f"ztmp{h}_{half}")
                    with nc.allow_low_precision(
                            reason="1/Z broadcast via f32r matmul"):
                        nc.vector.tensor_copy(ztmp[DH:HC, :], pout[DH:HC, :])
                        nc.vector.reciprocal(ztmp[DH:HC, :], ztmp[DH:HC, :])
                    ztmps[(h, half)] = ztmp

                def emit_d(e, half, act_ok):
                    # output projection yT[e-tile, q-half], transposed
                    g = e * 2 + half
                    pe_t = ps.tile([P, 1024], F32,
                                   tag="pp" if g % 2 == 0 else "sc",
                                   name=f"pe{e}_{half}")
                    for t in range(TD):
                        for cs in range(2):
                            c0 = half * 1024 + cs * BANK
                            nc.tensor.matmul(
                                pe_t[:, cs * BANK:(cs + 1) * BANK],
                                wo_s[t][:, e * P:(e + 1) * P],
                                xtu[t][:, c0:c0 + BANK],
                                start=(t == 0), stop=(t == TD - 1),
                            )
                    ot = outp.tile([P, 1024], F32, tag="ot", bufs=4,
                                   name=f"ot{e}_{half}")
                    if act_ok and g % 2 == 1:
                        nc.scalar.copy(ot[:], pe_t[:])
                    else:
                        nc.vector.tensor_copy(ot[:], pe_t[:])
                    nc.sync.dma_start(
                        y_t[e][:, half * 1024:(half + 1) * 1024], ot[:])

                biasq = bias_s["bq"] if has_bias else None
                biask = bias_s["bk"] if has_bias else None
                ws3 = {}
                if split:
                    # causal-style masks: q-half-0 attention uses only
                    # key-half-0, so project half-by-half with attention
                    # interleaved -- ScalarE exp hides the projections
                    for half in range(2):
                        if half == 0:
                            # interleave w and x DMAs so the first matmul's
                            # operands land early in the queues
                            ws3["q"], ws3["k"], ws3["v"] = [], [], []
                            xs = []
                            for i in range(KD):
                                ws3["q"].append(win.tile(
                                    [P, DG], BF16, tag="w", bufs=win_b,
                                    name=f"wq{i}"))
                                nc.sync.dma_start(ws3["q"][i][:], wq_t[i])
                                xt_ = xin.tile([P, 1024], BF16, tag="x",
                                               bufs=xin_b, name=f"xq0_{i}")
                                nc.sync.dma_start(xt_[:], xq_t[i][:, 0:1024])
                                xs.append(xt_)
                            xk0, xv0 = [], []
                            for i in range(KD):
                                ws3["k"].append(win.tile(
                                    [P, DG], BF16, tag="w", bufs=win_b,
                                    name=f"wk{i}"))
                                nc.sync.dma_start(ws3["k"][i][:], wk_t[i])
                                xt_ = xin.tile([P, 1024], BF16, tag="x",
                                               bufs=xin_b, name=f"xk0_{i}")
                                nc.sync.dma_start(xt_[:], xk_t[i][:, 0:1024])
                                xk0.append(xt_)
                            for i in range(KD):
                                ws3["v"].append(win.tile(
                                    [P, DG], BF16, tag="w", bufs=win_b,
                                    name=f"wv{i}"))
                                nc.sync.dma_start(ws3["v"][i][:], wv_t[i])
                                xt_ = xin.tile([P, 1024], BF16, tag="x",
                                               bufs=xin_b, name=f"xv0_{i}")
                                nc.sync.dma_start(xt_[:], xv_t[i][:, 0:1024])
                                xv0.append(xt_)
                            proj_qk(xs, ws3["q"], biasq, qt, "q", 0, 0)
                            proj_qk(xk0, ws3["k"], biask, kt, "k", 0, 0)
                            proj_v(xv0, range(8), 0)
                            late_loads()
                            # half-1 projection work interleaved into half-0
                            # attention (ScalarE-bound): V and dtiles 0-1
                            # here; dtiles 2-3 go into half-1 attention,
                            # which is also ScalarE-bound
                            units = []
                            xv1 = load_x8(xv_t, "v1", 1)
                            for j_ in range(8, NT):
                                units.append((proj_v, (xv1, [j_], 1024)))
                            xq1 = load_x8(xq_t, "q1", 1)
                            xk1 = load_x8(xk_t, "k1", 1)
                            units.append((proj_qk_t, (xq1, ws3["q"],
                                          biasq, qt, "q", 1, 0, 0)))
                            units.append((proj_qk_t, (xk1, ws3["k"],
                                          biask, kt, "k", 1, 0, 0)))
                            ui = 0
                            for h in range(HG):
                                emit_head_half(h, 0)
                                if h % 2 == 1 and h >= 3:
                                    emit_norm_half((h - 3) // 2, 0)
                                for _ in range(2 if h < 4 else 1):
                                    if ui < len(units):
                                        fn, args = units[ui]
                                        fn(*args)
                                        ui += 1
                            emit_norm_half(TD - 1, 0)
                            while ui < len(units):
                                fn, args = units[ui]
                                fn(*args)
                                ui += 1
                        else:
                            for h in range(HG):
                                if h in (1, 2, 4):
                                    t_ = {1: 1, 2: 2, 4: 3}[h]
                                    proj_qk_t(xq1, ws3["q"], biasq, qt,
                                              "q", 1, 0, t_)
                                    proj_qk_t(xk1, ws3["k"], biask, kt,
                                              "k", 1, 0, t_)
                                emit_head_half(h, 1)
                                if h % 2 == 1 and h >= 3:
                                    emit_norm_half((h - 3) // 2, 1)
                                # D's q-half-0 only needs the half-0 norms,
                                # which all completed in half-0 attention:
                                # fill half-1's PE idle with these groups
                                emit_d(h, 0, act_ok=False)
                            # first two output-projection groups' t<3
                            # accumulation depends only on xtu[0..2], so PE
                            # works through it while the pair-3 Z chain
                            # (ztmp copy -> recip -> zb) completes
                            pre = []
                            for e in range(2):
                                pe_t = ps.tile(
                                    [P, 1024], F32,
                                    tag="pp" if e % 2 == 0 else "sc",
                                    name=f"pe{e}_1")
                                for t in range(TD - 1):
                                    for cs in range(2):
                                        c0 = 1024 + cs * BANK
                                        nc.tensor.matmul(
                                            pe_t[:, cs * BANK:(cs + 1) * BANK],
                                            wo_s[t][:, e * P:(e + 1) * P],
                                            xtu[t][:, c0:c0 + BANK],
                                            start=(t == 0), stop=False,
                                        )
                                pre.append(pe_t)
                            emit_norm_half(TD - 1, 1)
                            for e in range(2):
                                pe_t = pre[e]
                                for cs in range(2):
                                    c0 = 1024 + cs * BANK
                                    nc.tensor.matmul(
                                        pe_t[:, cs * BANK:(cs + 1) * BANK],
                                        wo_s[TD - 1][:, e * P:(e + 1) * P],
                                        xtu[TD - 1][:, c0:c0 + BANK],
                                        start=False, stop=True,
                                    )
                                ot = outp.tile([P, 1024], F32, tag="ot",
                                               bufs=4, name=f"otp{e}_1")
                                if e % 2 == 1:
                                    nc.scalar.copy(ot[:], pe_t[:])
                                else:
                                    nc.vector.tensor_copy(ot[:], pe_t[:])
                                nc.sync.dma_start(y_t[e][:, 1024:2048], ot[:])
                            for e in range(2, KD):
                                emit_d(e, 1, act_ok=True)
                else:
                    xs = load_x8(xq_t, "q", None)
                    ws3["q"] = load_w8(wq_t, "q")
                    proj_qk(xs, ws3["q"], biasq, qt, "q", 0, 0)
                    proj_qk(xs, ws3["q"], biasq, qt, "q", 1, 1024)
                    xs = load_x8(xk_t, "k", None)
                    ws3["k"] = load_w8(wk_t, "k")
                    proj_qk(xs, ws3["k"], biask, kt, "k", 0, 0)
                    proj_qk(xs, ws3["k"], biask, kt, "k", 1, 1024)
                    xs = load_x8(xv_t, "v", None)
                    ws3["v"] = load_w8(wv_t, "v")
                    proj_v(xs, range(NT), 0)
                    late_loads()
                    for h in range(HG):
                        for half in range(2):
                            emit_head_half(h, half)
                            if h % 2 == 1 and h >= 3:
                                emit_norm_half((h - 3) // 2, half)
                    emit_norm_half(TD - 1, 0)
                    emit_norm_half(TD - 1, 1)
                    for e in range(KD):
                        for half in range(2):
                            emit_d(e, half, act_ok=True)

    nc.compile()
    return nc


def _get_nc(mask_bool, has_bias):
    key = (hash(mask_bool.tobytes()), has_bias)
    if key not in _cache:
        plan = _plan_from_mask(mask_bool, has_bias)
        _cache[key] = (_build(plan), plan)
    return _cache[key]


def kernel(query, key, value, mask, Wq, bq, Wk, bk, Wv, bv, Wo, bo):
    global _last_results
    bf = ml_dtypes.bfloat16
    query = np.asarray(query, dtype=np.float32)
    key = np.asarray(key, dtype=np.float32)
    value = np.asarray(value, dtype=np.float32)
    Wq = np.asarray(Wq, dtype=np.float32)
    Wk = np.asarray(Wk, dtype=np.float32)
    Wv = np.asarray(Wv, dtype=np.float32)
    Wo = np.asarray(Wo, dtype=np.float32)
    bq = np.asarray(bq, dtype=np.float32)
    bk = np.asarray(bk, dtype=np.float32)
    bv = np.asarray(bv, dtype=np.float32)
    bo = np.asarray(bo, dtype=np.float32)
    mask_bool = np.asarray(mask).reshape(S, S) != 0
    has_bias = bool(np.any(bq) or np.any(bk) or np.any(bv))

    nc, plan = _get_nc(mask_bool, has_bias)

    # head-parity selectors for the 1/Z broadcast matmuls: row 64 (the
    # partition the Z rows live on), column block hh selects the d-columns
    # of head-parity hh within a pair's 128-row dtile
    ind = np.zeros((HC, 2 * P), np.float32)
    for hh in range(2):
        for m in range(P):
            if m // DH == hh:
                ind[DH, hh * P + m] = 1.0

    nmix = max(1, len(plan["mixed_tiles"]))
    mm = np.zeros((nmix, P, P), bf)
    for idx, (i, j) in enumerate(plan["mixed_tiles"]):
        mm[idx] = mask_bool[i * P:(i + 1) * P, j * P:(j + 1) * P].T.astype(bf)

    in_maps = []
    for c in range(8):
        b, g = c // 2, c % 2
        gc = slice(g * DG, (g + 1) * DG)
        im = {
            "xq": query[b].T.astype(bf),
            "xk": key[b].T.astype(bf),
            "xv": value[b].T.astype(bf),
            "wq": Wq[:, gc].astype(bf),
            "wk": Wk[:, gc].astype(bf),
            "wv": Wv[:, gc].astype(bf),
            "wo": Wo[gc, :].astype(bf),
            "ind": ind,
            "mmask": mm,
        }
        if has_bias:
            im["bq"] = bq[gc].reshape(1, DG).astype(bf)
            im["bk"] = bk[gc].reshape(1, DG).astype(bf)
            im["bv"] = bv[gc].reshape(1, DG).astype(bf)
        in_maps.append(im)

    global _last_in_maps
    _last_in_maps = in_maps
    res = bass_utils.run_bass_kernel_spmd(nc, in_maps, core_ids=list(range(8)))
    _last_results = res

    out = np.empty((B, S, D), np.float32)
    for b in range(B):
        yT = res.results[2 * b]["yT"] + res.results[2 * b + 1]["yT"]
        out[b] = yT.T + bo
    return out



# revision 16
# speedup vs baseline: 1.1202x; 1.1202x over previous
"""Multi-head attention (B=4, S=2048, D=1024, H=16, DH=64) on 8 Trainium2
NeuronCores.

Sharding: core c handles batch b = c//2 and head-group g = c%2 (8 heads,
i.e. columns 512g:512(g+1) of Wq/Wk/Wv and rows 512g:512(g+1) of Wo).
Each core produces a partial output projection; the host sums the two
partials per batch and adds bo. No collectives.

Device kernel (per core, everything bf16 with fp32 PSUM accumulation):
  A. QT = Wq_g^T @ xq^T   [512, 2048]   (likewise KT), V = xv @ Wv_g
     stored interleaved with a ones column per head ("vext").
  B. Per head h, per key-tile j: scoresT[k, q] = KT_j^T-stationary matmul,
     exp via ScalarE (scale=1/sqrt(DH) folded in, no max subtraction -- the
     scores are bounded), causal/masked tiles handled by host-computed tile
     classification (skip / elementwise-multiply).  PV matmul with
     lhsT = [V | ones] accumulates unnormalized outT plus the softmax
     denominators Z in one pass.
  C. Normalize: xT *= broadcast(1/Z) (indicator-matrix matmul broadcast).
  D. yT = Wo_g-stationary projection of xT, written transposed; host
     re-transposes.
"""

import numpy as np
import ml_dtypes

import concourse.bacc as bacc
import concourse.mybir as mybir
import concourse.tile as tile
from concourse import bass_utils

BF16 = mybir.dt.bfloat16
F32 = mybir.dt.float32
F32R = mybir.dt.float32r
FP8 = mybir.dt.float8e4
DR = mybir.MatmulPerfMode.DoubleRow
EXP = mybir.ActivationFunctionType.Exp

B, S, D, H, DH = 4, 2048, 1024, 16, 64
P = 128
NT = S // P            # 16 key/query tiles
GROUPS = 2             # head groups (tensor parallel)
HG = H // GROUPS       # 8 heads per core
DG = D // GROUPS       # 512
KD = D // P            # 8 contraction tiles over D
KP = KD // 2           # 4 DoubleRow contraction pairs over D
TD = DG // P           # 4 d-tiles per group
HC = DH + 1            # 65: V columns + ones column per head
# fp8 weight scale: W*0.02 is subnormal in e4m3; scale by 16 so weights
# land in the normal range.  q/k come out 16x too big (exp scale folds
# 1/256), V comes out 16x too big (the vext "ones" column is 16 so the
# computed 1/Z absorbs it exactly -- powers of two, no rounding).
WS = 16.0
SCALE = float(DH) ** -0.5 / (WS * WS)
BANK = 512             # fp32 PSUM bank, in elements
MAX_PRELOAD_MASK = 64

_cache = {}
_last_results = None


def _plan_from_mask(mask_bool, has_bias):
    g = mask_bool.reshape(NT, P, NT, P).sum(axis=(1, 3))
    full = g == P * P
    zero = g == 0

    mixed_tiles = []        # ordered list of (i, j)
    mixed_of = {}           # (i, j) -> index into mixed_tiles
    j_info = [None] * NT
    for j in range(NT):
        act = [i for i in range(NT) if not zero[i, j]]
        if not act:
            continue
        i0, i1 = min(act), max(act) + 1
        mixed = []
        for i in range(i0, i1):
            if not full[i, j]:
                if (i, j) not in mixed_of:
                    mixed_of[(i, j)] = len(mixed_tiles)
                    mixed_tiles.append((i, j))
                mixed.append((i, mixed_of[(i, j)]))
        qlo, qhi = i0 * P, i1 * P
        # one slab per 1024-wide q-half; slab PSUM tile base s0a is
        # 512-aligned so the 512-aligned matmul chunks never cross a bank
        # inside the tile.
        slabs = {}
        for half in range(2):
            qb = max(qlo, half * 1024)
            qe = min(qhi, (half + 1) * 1024)
            if qb >= qe:
                continue
            s0a = (qb // BANK) * BANK
            chunks = []
            d = qb
            while d < qe:
                d2 = min((d // BANK + 1) * BANK, qe)
                chunks.append((d, d2))
                d = d2
            slabs[half] = (s0a, qb, qe, chunks)
        j_info[j] = dict(qlo=qlo, qhi=qhi, slabs=slabs, mixed=mixed)

    first_j = {}
    last_j = {}
    for j in range(NT):
        if j_info[j] is None:
            continue
        for (_, _, _, chunks) in j_info[j]["slabs"].values():
            for (c0, _) in chunks:
                bk = c0 // BANK
                first_j.setdefault(bk, j)
                last_j[bk] = j
    # split-schedule legality: q-half-0 attention touches only key-half-0
    # (true for causal), so projections can be computed half-by-half with
    # attention interleaved between them
    split = all(j_info[j] is None or 0 not in j_info[j]["slabs"]
                for j in range(NT // 2, NT))
    return dict(
        j_info=j_info,
        mixed_tiles=mixed_tiles,
        first_j=first_j,
        last_j=last_j,
        has_bias=has_bias,
        split=split,
    )


def _build(plan):
    has_bias = plan["has_bias"]
    j_info = plan["j_info"]
    mixed_tiles = plan["mixed_tiles"]
    nm = max(1, len(mixed_tiles))
    preload = len(mixed_tiles) <= MAX_PRELOAD_MASK

    nc = bacc.Bacc("TRN2", target_bir_lowering=False, debug=False)
    xq_d = nc.dram_tensor("xq", [D, S], FP8, kind="ExternalInput").ap()
    xk_d = nc.dram_tensor("xk", [D, S], FP8, kind="ExternalInput").ap()
    xv_d = nc.dram_tensor("xv", [D, S], BF16, kind="ExternalInput").ap()
    wq_d = nc.dram_tensor("wq", [D, DG], FP8, kind="ExternalInput").ap()
    wk_d = nc.dram_tensor("wk", [D, DG], FP8, kind="ExternalInput").ap()
    wv_d = nc.dram_tensor("wv", [D, DG], BF16, kind="ExternalInput").ap()
    wo_d = nc.dram_tensor("wo", [DG, D], BF16, kind="ExternalInput").ap()
    ind_d = nc.dram_tensor("ind", [HC, 2 * P], F32R, kind="ExternalInput").ap()
    mm_d = nc.dram_tensor("mmask", [nm, P, P], BF16, kind="ExternalInput").ap()
    if has_bias:
        bq_d = nc.dram_tensor("bq", [1, DG], BF16, kind="ExternalInput").ap()
        bk_d = nc.dram_tensor("bk", [1, DG], BF16, kind="ExternalInput").ap()
        bv_d = nc.dram_tensor("bv", [1, DG], BF16, kind="ExternalInput").ap()
    y_d = nc.dram_tensor("yT", [D, S], F32, kind="ExternalOutput").ap()

    # DoubleRow pair views: contraction row (2i*128+s*128+p) -> [i][p, s, .]
    xq_t = xq_d.rearrange("(n s p) q -> n p s q", p=P, s=2)
    xk_t = xk_d.rearrange("(n s p) q -> n p s q", p=P, s=2)
    xv_t = xv_d.rearrange("(n p) q -> n p q", p=P)
    wq_t = wq_d.rearrange("(n s p) d -> n p s d", p=P, s=2)
    wk_t = wk_d.rearrange("(n s p) d -> n p s d", p=P, s=2)
    wv_t = wv_d.rearrange("(n p) d -> n p d", p=P)
    wo_t = wo_d.rearrange("(n p) e -> n p e", p=P)
    y_t = y_d.rearrange("(n p) q -> n p q", p=P)

    with tile.TileContext(nc, trace_sim=False) as tc:
        with (
            tc.tile_pool(name="pers", bufs=1) as pers,
            tc.tile_pool(name="xin", bufs=12) as xin,
            tc.tile_pool(name="win", bufs=10) as win,
            tc.tile_pool(name="ptp", bufs=4) as ptp,
            tc.tile_pool(name="tmpp", bufs=1) as tmpp,
            tc.tile_pool(name="outp", bufs=2) as outp,
        ):
            # ---- persistent SBUF tensors -------------------------------
            qt = [pers.tile([P, S], BF16, tag="qt", bufs=TD, name=f"qt{t}")
                  for t in range(TD)]
            kt = [pers.tile([P, S], BF16, tag="kt", bufs=TD, name=f"kt{t}")
                  for t in range(TD)]
            vx = [pers.tile([P, HG * HC], BF16, tag="vx", bufs=NT,
                            name=f"vx{j}") for j in range(NT)]
            xtu = [pers.tile([P, S], BF16, tag="xtu", bufs=TD, name=f"xtu{t}")
                   for t in range(TD)]
            # head-parity selector rows for the 1/Z broadcast matmuls; row 64
            # so the base partition matches the ztmp Z-rows (bass requires
            # equal lhsT/rhs base partitions)
            ind_s = pers.tile([HC, 2 * P], F32R, tag="ind", bufs=1,
                              name="ind_s")
            wo_s = [pers.tile([P, D], BF16, tag="wo", bufs=TD, name=f"wo{t}")
                    for t in range(TD)]

            mtile = {}
            if preload:
                for idx, (i, j) in enumerate(mixed_tiles):
                    mtile[(i, j)] = pers.tile([P, P], BF16, tag="mt", bufs=nm,
                                              name=f"mt{idx}")

            if has_bias:
                ones = pers.tile([1, BANK], BF16, tag="ones", bufs=1,
                                 name="ones")
                nc.vector.memset(ones[:], 1.0)
                bias_s = {}
                for nm_, d_ in (("bq", bq_d), ("bk", bk_d), ("bv", bv_d)):
                    bs = pers.tile([1, DG], BF16, tag="bias", bufs=3,
                                   name=f"{nm_}_s")
                    nc.sync.dma_start(bs[:], d_)
                    bias_s[nm_] = bs

            # ---- Phases A+B share one PSUM pool ------------------------
            #   tag "pp" (2x2 banks): projection psums (A), pout tiles (B)
            #   tag "sc" (2x2 banks): V-proj psums, score slabs, zb tiles
            # Sharing tags across phases keeps the slots flowing with no
            # pool-boundary barrier, so V-proj overlaps early attention.
            with tc.tile_pool(name="psAB", bufs=2, space="PSUM") as ps:
                split = plan["split"]
                xin_b = 26 if split else 12
                win_b = 24 if split else 16
                ztmp_b = 6 if split else 8

                def load_w8(wd, label):
                    ts_ = []
                    for i in range(KP):
                        wt_ = win.tile([P, 2, DG], FP8, tag="w", bufs=win_b,
                                       name=f"w{label}{i}")
                        nc.sync.dma_start(wt_[:], wd[i])
                        ts_.append(wt_)
                    return ts_

                def load_x8(xd, label, half):
                    # half=None: full rows (serial); else one 1024-col half
                    w = S if half is None else 1024
                    off = 0 if half is None else 1024 * half
                    ts_ = []
                    for i in range(KP):
                        xt_ = xin.tile([P, 2, w], FP8, tag="x", bufs=xin_b,
                                       name=f"x{label}{i}")
                        nc.sync.dma_start(xt_[:], xd[i][:, :, off:off + w])
                        ts_.append(xt_)
                    return ts_

                def load_x8_bf(xd, label, half):
                    w = S if half is None else 1024
                    off = 0 if half is None else 1024 * half
                    ts_ = []
                    for i in range(KD):
                        xt_ = xin.tile([P, w], BF16, tag="x", bufs=xin_b,
                                       name=f"x{label}{i}")
                        nc.sync.dma_start(xt_[:], xd[i][:, off:off + w])
                        ts_.append(xt_)
                    return ts_

                def proj_qk_t(xs, ws, bias, out_tiles, label, half, xoff, t):
                    # out_tiles[t][:, half cols] = sum_i ws[i][:,:,t]^T @ xs[i]
                    pp = ps.tile([P, 1024], F32, tag="pp",
                                 name=f"ps_{label}{t}_{half}")
                    for i in range(KP):
                        for cs in range(2):
                            x0 = xoff + cs * BANK
                            nc.tensor.matmul(
                                pp[:, cs * BANK:(cs + 1) * BANK],
                                ws[i][:, :, t * P:(t + 1) * P],
                                xs[i][:, :, x0:x0 + BANK],
                                start=(i == 0),
                                stop=(i == KP - 1 and bias is None),
                                perf_mode=DR,
                            )
                    if bias is not None:
                        for cs in range(2):
                            nc.tensor.matmul(
                                pp[:, cs * BANK:(cs + 1) * BANK],
                                bias[0:1, t * P:(t + 1) * P],
                                ones[0:1, :],
                                start=False, stop=(cs == 1),
                            )
                    nc.vector.tensor_copy(
                        out_tiles[t][:, half * 1024:(half + 1) * 1024],
                        pp[:],
                    )

                def proj_qk(xs, ws, bias, out_tiles, label, half, xoff):
                    for t in range(TD):
                        proj_qk_t(xs, ws, bias, out_tiles, label, half,
                                  xoff, t)

                def proj_v(xs, jrange, xoff_base):
                    for j in jrange:
                        lc = j * P - xoff_base
                        psv = ps.tile([P, DG], F32, tag="pp", name=f"ps_v{j}")
                        for i in range(KD):
                            nc.tensor.matmul(
                                psv[:],
                                xs[i][:, lc:lc + P],
                                ws3["v"][i][:],
                                start=(i == 0),
                                stop=(i == KD - 1 and not has_bias),
                            )
                        if has_bias:
                            nc.tensor.matmul(
                                psv[:], ones[0:1, 0:P], bias_s["bv"][0:1, :],
                                start=False, stop=True,
                            )
                        vxv = vx[j][:].rearrange("p (g c) -> p g c", c=HC)
                        nc.vector.memset(vxv[:, :, DH:HC], 1.0)
                        nc.vector.tensor_copy(
                            vxv[:, :, 0:DH],
                            psv[:].rearrange("p (g c) -> p g c", c=DH),
                        )

                def late_loads():
                    # needed only from phase B onward; emitted after the x/w
                    # loads so they queue behind them on the DMA engines
                    nc.sync.dma_start(ind_s[:], ind_d)
                    for t in range(TD):
                        nc.sync.dma_start(wo_s[t][:], wo_t[t])
                    if preload:
                        for idx, (i, j) in enumerate(mixed_tiles):
                            nc.sync.dma_start(mtile[(i, j)][:], mm_d[idx])

                ztmps = {}

                def emit_norm_half(t, half):
                    # normalize xtu[t] q-half by 1/Z of head pair (2t, 2t+1)
                    zb = ps.tile([P, 1024], F32, tag="pp",
                                 name=f"zb{t}_{half}")
                    for hh in range(2):
                        zt_ = ztmps[(2 * t + hh, half)]
                        for cs in range(2):
                            nc.tensor.matmul(
                                zb[:, cs * BANK:(cs + 1) * BANK],
                                ind_s[DH:HC, hh * P:(hh + 1) * P],
                                zt_[DH:HC, cs * BANK:(cs + 1) * BANK],
                                start=(hh == 0), stop=(hh == 1),
                            )
                    nc.vector.tensor_mul(
                        xtu[t][:, half * 1024:(half + 1) * 1024],
                        xtu[t][:, half * 1024:(half + 1) * 1024],
                        zb[:],
                    )

                def emit_head_half(h, half, mid=None):
                    # `mid` = filler work (projection units, deferred norms,
                    # output-projection groups) emitted after the 4th key
                    # tile: mid-head DVE is idle, so the fillers' PSUM slots
                    # release promptly instead of queueing behind the
                    # head-boundary copy burst and starving ScalarE
                    t, r0 = h // 2, DH * (h % 2)
                    h0, h1 = half * 1024, (half + 1) * 1024
                    pout_t = ps.tile([P, 1024], F32, tag="pp",
                                     name=f"pout{h}_{half}")
                    pout = pout_t[0:HC]
                    wrote = False
                    for j in range(NT):
                        info = j_info[j]
                        if info is None or half not in info["slabs"]:
                            continue
                        (s0, qb, s1, chunks) = info["slabs"][half]
                        ps_s = ps.tile([P, 1024], F32, tag="sc",
                                       name=f"sc{h}_{j}_{half}")
                        for (c0, c1) in chunks:
                            nc.tensor.matmul(
                                ps_s[:, c0 - s0:c1 - s0],
                                kt[t][r0:r0 + DH, j * P:(j + 1) * P],
                                qt[t][r0:r0 + DH, c0:c1],
                                start=True, stop=True,
                            )
                        pt = ptp.tile([P, 1024], BF16, tag="pt", bufs=4,
                                      name=f"pt{h}_{j}_{half}")
                        nc.scalar.activation(
                            pt[:, qb - s0:s1 - s0],
                            ps_s[:, qb - s0:s1 - s0], EXP,
                            scale=SCALE,
                        )
                        for (i, idx) in info["mixed"]:
                            ic = i * P
                            if not (qb <= ic < s1):
                                continue
                            if preload:
                                mt = mtile[(i, j)]
                            else:
                                mt = ptp.tile([P, P], BF16, tag="mts",
                                              bufs=4, name=f"mts{h}_{j}_{i}")
                                nc.sync.dma_start(mt[:], mm_d[idx])
                            nc.vector.tensor_mul(
                                pt[:, ic - s0:ic - s0 + P],
                                pt[:, ic - s0:ic - s0 + P],
                                mt[:],
                            )
                        for (c0, c1) in chunks:
                            bk_ = c0 // BANK
                            nc.tensor.matmul(
                                pout[:, c0 - h0:c1 - h0],
                                vx[j][:, h * HC:(h + 1) * HC],
                                pt[:, c0 - s0:c1 - s0],
                                start=(j == plan["first_j"][bk_]),
                                stop=(j == plan["last_j"][bk_]),
                            )
                        wrote = True
                    if mid:
                        for fn_, args_ in mid:
                            fn_(*args_)
                    if not wrote:
                        return
                    # copy unnormalized head output + denominators out
                    if r0 == 0:
                        nc.vector.tensor_copy(xtu[t][0:DH, h0:h1],
                                              pout[0:DH, :])
                    else:
                        xtmp = tmpp.tile([DH, 1024], BF16, tag="xtmp",
                                         bufs=2, name=f"xtmp{h}_{half}")
                        nc.vector.tensor_copy(xtmp[:], pout[0:DH, :])
                        nc.sync.dma_start(xtu[t][DH:P, h0:h1], xtmp[:])
                    ztmp = tmpp.tile([HC, 1024], F32R, tag="ztmp",
                                     bufs=ztmp_b, name=f"ztmp{h}_{half}")
                    with nc.allow_low_precision(
                            reason="1/Z broadcast via f32r matmul"):
                        nc.vector.tensor_copy(ztmp[DH:HC, :], pout[DH:HC, :])
                        nc.vector.reciprocal(ztmp[DH:HC, :], ztmp[DH:HC, :])
                    ztmps[(h, half)] = ztmp

                def emit_d(e, half, act_ok):
                    # output projection yT[e-tile, q-half], transposed
                    g = e * 2 + half
                    pe_t = ps.tile([P, 1024], F32,
                                   tag="pp" if g % 2 == 0 else "sc",
                                   name=f"pe{e}_{half}")
                    for t in range(TD):
                        for cs in range(2):
                            c0 = half * 1024 + cs * BANK
                            nc.tensor.matmul(
                                pe_t[:, cs * BANK:(cs + 1) * BANK],
                                wo_s[t][:, e * P:(e + 1) * P],
                                xtu[t][:, c0:c0 + BANK],
                                start=(t == 0), stop=(t == TD - 1),
                            )
                    ot = outp.tile([P, 1024], F32, tag="ot", bufs=4,
                                   name=f"ot{e}_{half}")
                    if act_ok and g % 2 == 1:
                        nc.scalar.copy(ot[:], pe_t[:])
                    else:
                        nc.vector.tensor_copy(ot[:], pe_t[:])
                    nc.sync.dma_start(
                        y_t[e][:, half * 1024:(half + 1) * 1024], ot[:])

                biasq = bias_s["bq"] if has_bias else None
                biask = bias_s["bk"] if has_bias else None
                ws3 = {}
                if split:
                    # causal-style masks: q-half-0 attention uses only
                    # key-half-0, so project half-by-half with attention
                    # interleaved -- ScalarE exp hides the projections
                    for half in range(2):
                        if half == 0:
                            # interleave w and x DMAs so the first matmul's
                            # operands land early in the queues
                            ws3["q"], ws3["k"], ws3["v"] = [], [], []
                            xs = []
                            for i in range(KP):
                                ws3["q"].append(win.tile(
                                    [P, 2, DG], FP8, tag="w", bufs=win_b,
                                    name=f"wq{i}"))
                                nc.sync.dma_start(ws3["q"][i][:], wq_t[i])
                                xt_ = xin.tile([P, 2, 1024], FP8, tag="x",
                                               bufs=xin_b, name=f"xq0_{i}")
                                nc.sync.dma_start(xt_[:], xq_t[i][:, :, 0:1024])
                                xs.append(xt_)
                            xk0, xv0 = [], []
                            for i in range(KP):
                                ws3["k"].append(win.tile(
                                    [P, 2, DG], FP8, tag="w", bufs=win_b,
                                    name=f"wk{i}"))
                                nc.sync.dma_start(ws3["k"][i][:], wk_t[i])
                                xt_ = xin.tile([P, 2, 1024], FP8, tag="x",
                                               bufs=xin_b, name=f"xk0_{i}")
                                nc.sync.dma_start(xt_[:], xk_t[i][:, :, 0:1024])
                                xk0.append(xt_)
                            for i in range(KD):
                                ws3["v"].append(win.tile(
                                    [P, DG], BF16, tag="w", bufs=win_b,
                                    name=f"wv{i}"))
                                nc.sync.dma_start(ws3["v"][i][:], wv_t[i])
                                xt_ = xin.tile([P, 1024], BF16, tag="x",
                                               bufs=xin_b, name=f"xv0_{i}")
                                nc.sync.dma_start(xt_[:], xv_t[i][:, 0:1024])
                                xv0.append(xt_)
                            proj_qk(xs, ws3["q"], biasq, qt, "q", 0, 0)
                            proj_qk(xk0, ws3["k"], biask, kt, "k", 0, 0)
                            proj_v(xv0, range(8), 0)
                            late_loads()
                            # half-1 projection work interleaved into half-0
                            # attention (ScalarE-bound): V and dtiles 0-1
                            # here; dtiles 2-3 go into half-1 attention,
                            # which is also ScalarE-bound
                            units = []
                            xv1 = load_x8_bf(xv_t, "v1", 1)
                            for j_ in range(8, NT):
                                units.append((proj_v, (xv1, [j_], 1024)))
                            xq1 = load_x8(xq_t, "q1", 1)
                            xk1 = load_x8(xk_t, "k1", 1)
                            units.append((proj_qk_t, (xq1, ws3["q"],
                                          biasq, qt, "q", 1, 0, 0)))
                            units.append((proj_qk_t, (xk1, ws3["k"],
                                          biask, kt, "k", 1, 0, 0)))
                            ui = 0
                            for h in range(HG):
                                emit_head_half(h, 0)
                                if h % 2 == 1 and h >= 3:
                                    emit_norm_half((h - 3) // 2, 0)
                                for _ in range(2 if h < 4 else 1):
                                    if ui < len(units):
                                        fn, args = units[ui]
                                        fn(*args)
                                        ui += 1
                            emit_norm_half(TD - 1, 0)
                            while ui < len(units):
                                fn, args = units[ui]
                                fn(*args)
                                ui += 1
                        else:
                            for h in range(HG):
                                if h in (1, 2, 4):
                                    t_ = {1: 1, 2: 2, 4: 3}[h]
                                    proj_qk_t(xq1, ws3["q"], biasq, qt,
                                              "q", 1, 0, t_)
                                    proj_qk_t(xk1, ws3["k"], biask, kt,
                                              "k", 1, 0, t_)
                                emit_head_half(h, 1)
                                if h % 2 == 1 and h >= 3:
                                    emit_norm_half((h - 3) // 2, 1)
                                # D's q-half-0 only needs the half-0 norms,
                                # which all completed in half-0 attention:
                                # fill half-1's PE idle with these groups
                                emit_d(h, 0, act_ok=False)
                            # first two output-projection groups' t<3
                            # accumulation depends only on xtu[0..2], so PE
                            # works through it while the pair-3 Z chain
                            # (ztmp copy -> recip -> zb) completes
                            pre = []
                            for e in range(2):
                                pe_t = ps.tile(
                                    [P, 1024], F32,
                                    tag="pp" if e % 2 == 0 else "sc",
                                    name=f"pe{e}_1")
                                for t in range(TD - 1):
                                    for cs in range(2):
                                        c0 = 1024 + cs * BANK
                                        nc.tensor.matmul(
                                            pe_t[:, cs * BANK:(cs + 1) * BANK],
                                            wo_s[t][:, e * P:(e + 1) * P],
                                            xtu[t][:, c0:c0 + BANK],
                                            start=(t == 0), stop=False,
                                        )
                                pre.append(pe_t)
                            emit_norm_half(TD - 1, 1)
                            for e in range(2):
                                pe_t = pre[e]
                                for cs in range(2):
                                    c0 = 1024 + cs * BANK
                                    nc.tensor.matmul(
                                        pe_t[:, cs * BANK:(cs + 1) * BANK],
                                        wo_s[TD - 1][:, e * P:(e + 1) * P],
                                        xtu[TD - 1][:, c0:c0 + BANK],
                                        start=False, stop=True,
                                    )
                                ot = outp.tile([P, 1024], F32, tag="ot",
                                               bufs=4, name=f"otp{e}_1")
                                if e % 2 == 1:
                                    nc.scalar.copy(ot[:], pe_t[:])
                                else:
                                    nc.vector.tensor_copy(ot[:], pe_t[:])
                                nc.sync.dma_start(y_t[e][:, 1024:2048], ot[:])
                            for e in range(2, KD):
                                emit_d(e, 1, act_ok=True)
                else:
                    xs = load_x8(xq_t, "q", None)
                    ws3["q"] = load_w8(wq_t, "q")
                    proj_qk(xs, ws3["q"], biasq, qt, "q", 0, 0)
                    proj_qk(xs, ws3["q"], biasq, qt, "q", 1, 1024)
                    xs = load_x8(xk_t, "k", None)
                    ws3["k"] = load_w8(wk_t, "k")
                    proj_qk(xs, ws3["k"], biask, kt, "k", 0, 0)
                    proj_qk(xs, ws3["k"], biask, kt, "k", 1, 1024)
                    xs = load_x8_bf(xv_t, "v", None)
                    ws3["v"] = []
                    for i in range(KD):
                        wt_ = win.tile([P, DG], BF16, tag="w", bufs=win_b,
                                       name=f"wvn{i}")
                        nc.sync.dma_start(wt_[:], wv_t[i])
                        ws3["v"].append(wt_)
                    proj_v(xs, range(NT), 0)
                    late_loads()
                    for h in range(HG):
                        for half in range(2):
                            emit_head_half(h, half)
                            if h % 2 == 1 and h >= 3:
                                emit_norm_half((h - 3) // 2, half)
                    emit_norm_half(TD - 1, 0)
                    emit_norm_half(TD - 1, 1)
                    for e in range(KD):
                        for half in range(2):
                            emit_d(e, half, act_ok=True)

    nc.compile()
    return nc


def _get_nc(mask_bool, has_bias):
    key = (hash(mask_bool.tobytes()), has_bias)
    if key not in _cache:
        plan = _plan_from_mask(mask_bool, has_bias)
        _cache[key] = (_build(plan), plan)
    return _cache[key]


def kernel(query, key, value, mask, Wq, bq, Wk, bk, Wv, bv, Wo, bo):
    global _last_results
    bf = ml_dtypes.bfloat16
    f8 = ml_dtypes.float8_e4m3
    query = np.asarray(query, dtype=np.float32)
    key = np.asarray(key, dtype=np.float32)
    value = np.asarray(value, dtype=np.float32)
    Wq = np.asarray(Wq, dtype=np.float32)
    Wk = np.asarray(Wk, dtype=np.float32)
    Wv = np.asarray(Wv, dtype=np.float32)
    Wo = np.asarray(Wo, dtype=np.float32)
    bq = np.asarray(bq, dtype=np.float32)
    bk = np.asarray(bk, dtype=np.float32)
    bv = np.asarray(bv, dtype=np.float32)
    bo = np.asarray(bo, dtype=np.float32)
    mask_bool = np.asarray(mask).reshape(S, S) != 0
    has_bias = bool(np.any(bq) or np.any(bk) or np.any(bv))

    nc, plan = _get_nc(mask_bool, has_bias)

    # head-parity selectors for the 1/Z broadcast matmuls: row 64 (the
    # partition the Z rows live on), column block hh selects the d-columns
    # of head-parity hh within a pair's 128-row dtile
    ind = np.zeros((HC, 2 * P), np.float32)
    for hh in range(2):
        for m in range(P):
            if m // DH == hh:
                ind[DH, hh * P + m] = 1.0

    nmix = max(1, len(plan["mixed_tiles"]))
    mm = np.zeros((nmix, P, P), bf)
    for idx, (i, j) in enumerate(plan["mixed_tiles"]):
        mm[idx] = mask_bool[i * P:(i + 1) * P, j * P:(j + 1) * P].T.astype(bf)

    in_maps = []
    xq_f8 = [query[b].T.astype(f8) for b in range(B)]
    xk_f8 = [key[b].T.astype(f8) for b in range(B)]
    xv_bf = [value[b].T.astype(bf) for b in range(B)]
    for c in range(8):
        b, g = c // 2, c % 2
        gc = slice(g * DG, (g + 1) * DG)
        im = {
            "xq": xq_f8[b],
            "xk": xk_f8[b],
            "xv": xv_bf[b],
            "wq": (Wq[:, gc] * WS).astype(f8),
            "wk": (Wk[:, gc] * WS).astype(f8),
            "wv": Wv[:, gc].astype(bf),
            "wo": Wo[gc, :].astype(bf),
            "ind": ind,
            "mmask": mm,
        }
        if has_bias:
            im["bq"] = (bq[gc] * WS).reshape(1, DG).astype(bf)
            im["bk"] = (bk[gc] * WS).reshape(1, DG).astype(bf)
            im["bv"] = bv[gc].reshape(1, DG).astype(bf)
        in_maps.append(im)

    global _last_in_maps
    _last_in_maps = in_maps
    res = bass_utils.run_bass_kernel_spmd(nc, in_maps, core_ids=list(range(8)))
    _last_results = res

    out = np.empty((B, S, D), np.float32)
    for b in range(B):
        yT = res.results[2 * b]["yT"] + res.results[2 * b + 1]["yT"]
        out[b] = yT.T + bo
    return out



# revision 24
# speedup vs baseline: 1.1239x; 1.0032x over previous
"""Multi-head attention (B=4, S=2048, D=1024, H=16, DH=64) on 8 Trainium2
NeuronCores.

Sharding: core c handles batch b = c//2 and head-group g = c%2 (8 heads,
i.e. columns 512g:512(g+1) of Wq/Wk/Wv and rows 512g:512(g+1) of Wo).
Each core produces a partial output projection; the host sums the two
partials per batch and adds bo. No collectives.

Device kernel (per core, everything bf16 with fp32 PSUM accumulation):
  A. QT = Wq_g^T @ xq^T   [512, 2048]   (likewise KT), V = xv @ Wv_g
     stored interleaved with a ones column per head ("vext").
  B. Per head h, per key-tile j: scoresT[k, q] = KT_j^T-stationary matmul,
     exp via ScalarE (scale=1/sqrt(DH) folded in, no max subtraction -- the
     scores are bounded), causal/masked tiles handled by host-computed tile
     classification (skip / elementwise-multiply).  PV matmul with
     lhsT = [V | ones] accumulates unnormalized outT plus the softmax
     denominators Z in one pass.
  C. Normalize: xT *= broadcast(1/Z) (indicator-matrix matmul broadcast).
  D. yT = Wo_g-stationary projection of xT, written transposed; host
     re-transposes.
"""

import numpy as np
import ml_dtypes

import concourse.bacc as bacc
import concourse.mybir as mybir
import concourse.tile as tile
from concourse import bass_utils

BF16 = mybir.dt.bfloat16
F32 = mybir.dt.float32
F32R = mybir.dt.float32r
FP8 = mybir.dt.float8e4
DR = mybir.MatmulPerfMode.DoubleRow
EXP = mybir.ActivationFunctionType.Exp

B, S, D, H, DH = 4, 2048, 1024, 16, 64
P = 128
NT = S // P            # 16 key/query tiles
GROUPS = 2             # head groups (tensor parallel)
HG = H // GROUPS       # 8 heads per core
DG = D // GROUPS       # 512
KD = D // P            # 8 contraction tiles over D
KP = KD // 2           # 4 DoubleRow contraction pairs over D
TD = DG // P           # 4 d-tiles per group
HC = DH + 1            # 65: V columns + ones column per head
# fp8 weight scale: W*0.02 is subnormal in e4m3; scale by 16 so weights
# land in the normal range.  q/k come out 16x too big (exp scale folds
# 1/256), V comes out 16x too big (the vext "ones" column is 16 so the
# computed 1/Z absorbs it exactly -- powers of two, no rounding).
WS = 16.0
SCALE = float(DH) ** -0.5 / (WS * WS)
BANK = 512             # fp32 PSUM bank, in elements
MAX_PRELOAD_MASK = 64
# folded q/k layout for fp8 DoubleRow scores: 3 heads per [128, 2, S]
# tile, head h -> tile h//3, partition quadrant 32*(h%3)+c, dh = 32*i+c
# (quadrant base 96 is not addressable, hence 3 heads/tile).  The q/k
# projections write 6 permuted 128-column blocks (tile, fold) each.
FB = 6                 # folded projection column blocks
DGF = FB * P           # 768: permuted q/k weight width (incl. pad)

_cache = {}
_last_results = None


def _plan_from_mask(mask_bool, has_bias):
    g = mask_bool.reshape(NT, P, NT, P).sum(axis=(1, 3))
    full = g == P * P
    zero = g == 0

    mixed_tiles = []        # ordered list of (i, j)
    mixed_of = {}           # (i, j) -> index into mixed_tiles
    j_info = [None] * NT
    for j in range(NT):
        act = [i for i in range(NT) if not zero[i, j]]
        if not act:
            continue
        i0, i1 = min(act), max(act) + 1
        mixed = []
        for i in range(i0, i1):
            if not full[i, j]:
                if (i, j) not in mixed_of:
                    mixed_of[(i, j)] = len(mixed_tiles)
                    mixed_tiles.append((i, j))
                mixed.append((i, mixed_of[(i, j)]))
        qlo, qhi = i0 * P, i1 * P
        # one slab per 1024-wide q-half; slab PSUM tile base s0a is
        # 512-aligned so the 512-aligned matmul chunks never cross a bank
        # inside the tile.
        slabs = {}
        for half in range(2):
            qb = max(qlo, half * 1024)
            qe = min(qhi, (half + 1) * 1024)
            if qb >= qe:
                continue
            s0a = (qb // BANK) * BANK
            chunks = []
            d = qb
            while d < qe:
                d2 = min((d // BANK + 1) * BANK, qe)
                chunks.append((d, d2))
                d = d2
            slabs[half] = (s0a, qb, qe, chunks)
        j_info[j] = dict(qlo=qlo, qhi=qhi, slabs=slabs, mixed=mixed)

    first_j = {}
    last_j = {}
    for j in range(NT):
        if j_info[j] is None:
            continue
        for (_, _, _, chunks) in j_info[j]["slabs"].values():
            for (c0, _) in chunks:
                bk = c0 // BANK
                first_j.setdefault(bk, j)
                last_j[bk] = j
    # split-schedule legality: q-half-0 attention touches only key-half-0
    # (true for causal), so projections can be computed half-by-half with
    # attention interleaved between them
    split = all(j_info[j] is None or 0 not in j_info[j]["slabs"]
                for j in range(NT // 2, NT))
    return dict(
        j_info=j_info,
        mixed_tiles=mixed_tiles,
        first_j=first_j,
        last_j=last_j,
        has_bias=has_bias,
        split=split,
    )


def _build(plan):
    has_bias = plan["has_bias"]
    j_info = plan["j_info"]
    mixed_tiles = plan["mixed_tiles"]
    nm = max(1, len(mixed_tiles))
    preload = len(mixed_tiles) <= MAX_PRELOAD_MASK

    nc = bacc.Bacc("TRN2", target_bir_lowering=False, debug=False)
    xq_d = nc.dram_tensor("xq", [D, S], FP8, kind="ExternalInput").ap()
    xk_d = nc.dram_tensor("xk", [D, S], FP8, kind="ExternalInput").ap()
    xv_d = nc.dram_tensor("xv", [D, S], BF16, kind="ExternalInput").ap()
    wq_d = nc.dram_tensor("wq", [D, DG], FP8, kind="ExternalInput").ap()
    wk_d = nc.dram_tensor("wk", [D, DG], FP8, kind="ExternalInput").ap()
    wv_d = nc.dram_tensor("wv", [D, DG], BF16, kind="ExternalInput").ap()
    wo_d = nc.dram_tensor("wo", [DG, D], BF16, kind="ExternalInput").ap()
    ind_d = nc.dram_tensor("ind", [HC, 2 * P], F32R, kind="ExternalInput").ap()
    mm_d = nc.dram_tensor("mmask", [nm, P, P], BF16, kind="ExternalInput").ap()
    if has_bias:
        bq_d = nc.dram_tensor("bq", [1, DG], BF16, kind="ExternalInput").ap()
        bk_d = nc.dram_tensor("bk", [1, DG], BF16, kind="ExternalInput").ap()
        bv_d = nc.dram_tensor("bv", [1, DG], BF16, kind="ExternalInput").ap()
    y_d = nc.dram_tensor("yT", [D, S], F32, kind="ExternalOutput").ap()

    # DoubleRow pair views: contraction row (2i*128+s*128+p) -> [i][p, s, .]
    xq_t = xq_d.rearrange("(n s p) q -> n p s q", p=P, s=2)
    xk_t = xk_d.rearrange("(n s p) q -> n p s q", p=P, s=2)
    xv_t = xv_d.rearrange("(n p) q -> n p q", p=P)
    wq_t = wq_d.rearrange("(n s p) d -> n p s d", p=P, s=2)
    wk_t = wk_d.rearrange("(n s p) d -> n p s d", p=P, s=2)
    wv_t = wv_d.rearrange("(n p) d -> n p d", p=P)
    wo_t = wo_d.rearrange("(n p) e -> n p e", p=P)
    y_t = y_d.rearrange("(n p) q -> n p q", p=P)

    with tile.TileContext(nc, trace_sim=False) as tc:
        with (
            tc.tile_pool(name="pers", bufs=1) as pers,
            tc.tile_pool(name="xin", bufs=12) as xin,
            tc.tile_pool(name="win", bufs=10) as win,
            tc.tile_pool(name="ptp", bufs=4) as ptp,
            tc.tile_pool(name="tmpp", bufs=1) as tmpp,
            tc.tile_pool(name="outp", bufs=2) as outp,
        ):
            # ---- persistent SBUF tensors -------------------------------
            qt = [pers.tile([P, S], BF16, tag="qt", bufs=TD, name=f"qt{t}")
                  for t in range(TD)]
            kt = [pers.tile([P, S], BF16, tag="kt", bufs=TD, name=f"kt{t}")
                  for t in range(TD)]
            vx = [pers.tile([P, HG * HC], BF16, tag="vx", bufs=NT,
                            name=f"vx{j}") for j in range(NT)]
            xtu = [pers.tile([P, S], BF16, tag="xtu", bufs=TD, name=f"xtu{t}")
                   for t in range(TD)]
            # head-parity selector rows for the 1/Z broadcast matmuls; row 64
            # so the base partition matches the ztmp Z-rows (bass requires
            # equal lhsT/rhs base partitions)
            ind_s = pers.tile([HC, 2 * P], F32R, tag="ind", bufs=1,
                              name="ind_s")
            wo_s = [pers.tile([P, D], BF16, tag="wo", bufs=TD, name=f"wo{t}")
                    for t in range(TD)]

            mtile = {}
            if preload:
                for idx, (i, j) in enumerate(mixed_tiles):
                    mtile[(i, j)] = pers.tile([P, P], BF16, tag="mt", bufs=nm,
                                              name=f"mt{idx}")

            if has_bias:
                ones = pers.tile([1, BANK], BF16, tag="ones", bufs=1,
                                 name="ones")
                nc.vector.memset(ones[:], 1.0)
                bias_s = {}
                for nm_, d_ in (("bq", bq_d), ("bk", bk_d), ("bv", bv_d)):
                    bs = pers.tile([1, DG], BF16, tag="bias", bufs=3,
                                   name=f"{nm_}_s")
                    nc.sync.dma_start(bs[:], d_)
                    bias_s[nm_] = bs

            # ---- Phases A+B share one PSUM pool ------------------------
            #   tag "pp" (2x2 banks): projection psums (A), pout tiles (B)
            #   tag "sc" (2x2 banks): V-proj psums, score slabs, zb tiles
            # Sharing tags across phases keeps the slots flowing with no
            # pool-boundary barrier, so V-proj overlaps early attention.
            with tc.tile_pool(name="psAB", bufs=2, space="PSUM") as ps:
                split = plan["split"]
                xin_b = 26 if split else 12
                win_b = 24 if split else 16
                ztmp_b = 6 if split else 8

                def load_w8(wd, label):
                    ts_ = []
                    for i in range(KP):
                        wt_ = win.tile([P, 2, DG], FP8, tag="w", bufs=win_b,
                                       name=f"w{label}{i}")
                        nc.sync.dma_start(wt_[:], wd[i])
                        ts_.append(wt_)
                    return ts_

                def load_x8(xd, label, half):
                    # half=None: full rows (serial); else one 1024-col half
                    w = S if half is None else 1024
                    off = 0 if half is None else 1024 * half
                    ts_ = []
                    for i in range(KP):
                        xt_ = xin.tile([P, 2, w], FP8, tag="x", bufs=xin_b,
                                       name=f"x{label}{i}")
                        nc.sync.dma_start(xt_[:], xd[i][:, :, off:off + w])
                        ts_.append(xt_)
                    return ts_

                def load_x8_bf(xd, label, half):
                    w = S if half is None else 1024
                    off = 0 if half is None else 1024 * half
                    ts_ = []
                    for i in range(KD):
                        xt_ = xin.tile([P, w], BF16, tag="x", bufs=xin_b,
                                       name=f"x{label}{i}")
                        nc.sync.dma_start(xt_[:], xd[i][:, off:off + w])
                        ts_.append(xt_)
                    return ts_

                def proj_qk_t(xs, ws, bias, out_tiles, label, half, xoff, t):
                    # out_tiles[t][:, half cols] = sum_i ws[i][:,:,t]^T @ xs[i]
                    pp = ps.tile([P, 1024], F32, tag="pp",
                                 name=f"ps_{label}{t}_{half}")
                    for i in range(KP):
                        for cs in range(2):
                            x0 = xoff + cs * BANK
                            nc.tensor.matmul(
                                pp[:, cs * BANK:(cs + 1) * BANK],
                                ws[i][:, :, t * P:(t + 1) * P],
                                xs[i][:, :, x0:x0 + BANK],
                                start=(i == 0),
                                stop=(i == KP - 1 and bias is None),
                                perf_mode=DR,
                            )
                    if bias is not None:
                        for cs in range(2):
                            nc.tensor.matmul(
                                pp[:, cs * BANK:(cs + 1) * BANK],
                                bias[0:1, t * P:(t + 1) * P],
                                ones[0:1, :],
                                start=False, stop=(cs == 1),
                            )
                    nc.vector.tensor_copy(
                        out_tiles[t][:, half * 1024:(half + 1) * 1024],
                        pp[:],
                    )

                def proj_qk(xs, ws, bias, out_tiles, label, half, xoff):
                    for t in range(TD):
                        proj_qk_t(xs, ws, bias, out_tiles, label, half,
                                  xoff, t)

                def proj_v(xs, jrange, xoff_base):
                    for j in jrange:
                        lc = j * P - xoff_base
                        psv = ps.tile([P, DG], F32, tag="pp", name=f"ps_v{j}")
                        for i in range(KD):
                            nc.tensor.matmul(
                                psv[:],
                                xs[i][:, lc:lc + P],
                                ws3["v"][i][:],
                                start=(i == 0),
                                stop=(i == KD - 1 and not has_bias),
                            )
                        if has_bias:
                            nc.tensor.matmul(
                                psv[:], ones[0:1, 0:P], bias_s["bv"][0:1, :],
                                start=False, stop=True,
                            )
                        vxv = vx[j][:].rearrange("p (g c) -> p g c", c=HC)
                        nc.vector.memset(vxv[:, :, DH:HC], 1.0)
                        nc.vector.tensor_copy(
                            vxv[:, :, 0:DH],
                            psv[:].rearrange("p (g c) -> p g c", c=DH),
                        )

                def late_loads():
                    # needed only from phase B onward; emitted after the x/w
                    # loads so they queue behind them on the DMA engines
                    nc.sync.dma_start(ind_s[:], ind_d)
                    for t in range(TD):
                        nc.sync.dma_start(wo_s[t][:], wo_t[t])
                    if preload:
                        for idx, (i, j) in enumerate(mixed_tiles):
                            nc.sync.dma_start(mtile[(i, j)][:], mm_d[idx])

                ztmps = {}

                def emit_norm_half(t, half):
                    # normalize xtu[t] q-half by 1/Z of head pair (2t, 2t+1)
                    zb = ps.tile([P, 1024], F32, tag="pp",
                                 name=f"zb{t}_{half}")
                    for hh in range(2):
                        zt_ = ztmps[(2 * t + hh, half)]
                        for cs in range(2):
                            nc.tensor.matmul(
                                zb[:, cs * BANK:(cs + 1) * BANK],
                                ind_s[DH:HC, hh * P:(hh + 1) * P],
                                zt_[DH:HC, cs * BANK:(cs + 1) * BANK],
                                start=(hh == 0), stop=(hh == 1),
                            )
                    nc.vector.tensor_mul(
                        xtu[t][:, half * 1024:(half + 1) * 1024],
                        xtu[t][:, half * 1024:(half + 1) * 1024],
                        zb[:],
                    )

                def emit_head_half(h, half, mid=None):
                    # `mid` = filler work (projection units, deferred norms,
                    # output-projection groups) emitted after the 4th key
                    # tile: mid-head DVE is idle, so the fillers' PSUM slots
                    # release promptly instead of queueing behind the
                    # head-boundary copy burst and starving ScalarE
                    t, r0 = h // 2, DH * (h % 2)
                    h0, h1 = half * 1024, (half + 1) * 1024
                    pout_t = ps.tile([P, 1024], F32, tag="pp",
                                     name=f"pout{h}_{half}")
                    pout = pout_t[0:HC]
                    wrote = False
                    for j in range(NT):
                        info = j_info[j]
                        if info is None or half not in info["slabs"]:
                            continue
                        (s0, qb, s1, chunks) = info["slabs"][half]
                        ps_s = ps.tile([P, 1024], F32, tag="sc",
                                       name=f"sc{h}_{j}_{half}")
                        for (c0, c1) in chunks:
                            nc.tensor.matmul(
                                ps_s[:, c0 - s0:c1 - s0],
                                kt[t][r0:r0 + DH, j * P:(j + 1) * P],
                                qt[t][r0:r0 + DH, c0:c1],
                                start=True, stop=True,
                            )
                        pt = ptp.tile([P, 1024], BF16, tag="pt", bufs=4,
                                      name=f"pt{h}_{j}_{half}")
                        nc.scalar.activation(
                            pt[:, qb - s0:s1 - s0],
                            ps_s[:, qb - s0:s1 - s0], EXP,
                            scale=SCALE,
                        )
                        for (i, idx) in info["mixed"]:
                            ic = i * P
                            if not (qb <= ic < s1):
                                continue
                            if preload:
                                mt = mtile[(i, j)]
                            else:
                                mt = ptp.tile([P, P], BF16, tag="mts",
                                              bufs=4, name=f"mts{h}_{j}_{i}")
                                nc.sync.dma_start(mt[:], mm_d[idx])
                            nc.gpsimd.tensor_mul(
                                pt[:, ic - s0:ic - s0 + P],
                                pt[:, ic - s0:ic - s0 + P],
                                mt[:],
                            )
                        for (c0, c1) in chunks:
                            bk_ = c0 // BANK
                            nc.tensor.matmul(
                                pout[:, c0 - h0:c1 - h0],
                                vx[j][:, h * HC:(h + 1) * HC],
                                pt[:, c0 - s0:c1 - s0],
                                start=(j == plan["first_j"][bk_]),
                                stop=(j == plan["last_j"][bk_]),
                            )
                        wrote = True
                    if mid:
                        for fn_, args_ in mid:
                            fn_(*args_)
                    if not wrote:
                        return
                    # copy unnormalized head output + denominators out
                    if r0 == 0:
                        nc.vector.tensor_copy(xtu[t][0:DH, h0:h1],
                                              pout[0:DH, :])
                    else:
                        xtmp = tmpp.tile([DH, 1024], BF16, tag="xtmp",
                                         bufs=2, name=f"xtmp{h}_{half}")
                        nc.vector.tensor_copy(xtmp[:], pout[0:DH, :])
                        nc.sync.dma_start(xtu[t][DH:P, h0:h1], xtmp[:])
                    ztmp = tmpp.tile([HC, 1024], F32R, tag="ztmp",
                                     bufs=ztmp_b, name=f"ztmp{h}_{half}")
                    with nc.allow_low_precision(
                            reason="1/Z broadcast via f32r matmul"):
                        nc.vector.reciprocal(ztmp[DH:HC, :], pout[DH:HC, :])
                    ztmps[(h, half)] = ztmp

                def emit_d(e, half, act_ok):
                    # output projection yT[e-tile, q-half], transposed
                    g = e * 2 + half
                    pe_t = ps.tile([P, 1024], F32,
                                   tag="pp" if g % 2 == 0 else "sc",
                                   name=f"pe{e}_{half}")
                    for t in range(TD):
                        for cs in range(2):
                            c0 = half * 1024 + cs * BANK
                            nc.tensor.matmul(
                                pe_t[:, cs * BANK:(cs + 1) * BANK],
                                wo_s[t][:, e * P:(e + 1) * P],
                                xtu[t][:, c0:c0 + BANK],
                                start=(t == 0), stop=(t == TD - 1),
                            )
                    ot = outp.tile([P, 1024], F32, tag="ot", bufs=4,
                                   name=f"ot{e}_{half}")
                    if act_ok and g % 2 == 1:
                        nc.scalar.copy(ot[:], pe_t[:])
                    else:
                        nc.vector.tensor_copy(ot[:], pe_t[:])
                    nc.sync.dma_start(
                        y_t[e][:, half * 1024:(half + 1) * 1024], ot[:])

                biasq = bias_s["bq"] if has_bias else None
                biask = bias_s["bk"] if has_bias else None
                ws3 = {}
                if split:
                    # causal-style masks: q-half-0 attention uses only
                    # key-half-0, so project half-by-half with attention
                    # interleaved -- ScalarE exp hides the projections
                    for half in range(2):
                        if half == 0:
                            # interleave w and x DMAs so the first matmul's
                            # operands land early in the queues
                            ws3["q"], ws3["k"], ws3["v"] = [], [], []
                            xs = []
                            for i in range(KP):
                                ws3["q"].append(win.tile(
                                    [P, 2, DG], FP8, tag="w", bufs=win_b,
                                    name=f"wq{i}"))
                                nc.sync.dma_start(ws3["q"][i][:], wq_t[i])
                                xt_ = xin.tile([P, 2, 1024], FP8, tag="x",
                                               bufs=xin_b, name=f"xq0_{i}")
                                nc.sync.dma_start(xt_[:], xq_t[i][:, :, 0:1024])
                                xs.append(xt_)
                            xk0, xv0 = [], []
                            for i in range(KP):
                                ws3["k"].append(win.tile(
                                    [P, 2, DG], FP8, tag="w", bufs=win_b,
                                    name=f"wk{i}"))
                                nc.sync.dma_start(ws3["k"][i][:], wk_t[i])
                                xt_ = xin.tile([P, 2, 1024], FP8, tag="x",
                                               bufs=xin_b, name=f"xk0_{i}")
                                nc.sync.dma_start(xt_[:], xk_t[i][:, :, 0:1024])
                                xk0.append(xt_)
                            for i in range(KD):
                                ws3["v"].append(win.tile(
                                    [P, DG], BF16, tag="w", bufs=win_b,
                                    name=f"wv{i}"))
                                nc.sync.dma_start(ws3["v"][i][:], wv_t[i])
                                xt_ = xin.tile([P, 1024], BF16, tag="x",
                                               bufs=xin_b, name=f"xv0_{i}")
                                nc.sync.dma_start(xt_[:], xv_t[i][:, 0:1024])
                                xv0.append(xt_)
                            proj_qk(xs, ws3["q"], biasq, qt, "q", 0, 0)
                            proj_qk(xk0, ws3["k"], biask, kt, "k", 0, 0)
                            proj_v(xv0, range(8), 0)
                            late_loads()
                            # half-1 projection work interleaved into half-0
                            # attention (ScalarE-bound): V and dtiles 0-1
                            # here; dtiles 2-3 go into half-1 attention,
                            # which is also ScalarE-bound
                            units = []
                            xv1 = load_x8_bf(xv_t, "v1", 1)
                            for j_ in range(8, NT):
                                units.append((proj_v, (xv1, [j_], 1024)))
                            xq1 = load_x8(xq_t, "q1", 1)
                            xk1 = load_x8(xk_t, "k1", 1)
                            units.append((proj_qk_t, (xq1, ws3["q"],
                                          biasq, qt, "q", 1, 0, 0)))
                            units.append((proj_qk_t, (xk1, ws3["k"],
                                          biask, kt, "k", 1, 0, 0)))
                            ui = 0
                            for h in range(HG):
                                emit_head_half(h, 0)
                                if h % 2 == 1 and h >= 3:
                                    emit_norm_half((h - 3) // 2, 0)
                                for _ in range(2 if h < 4 else 1):
                                    if ui < len(units):
                                        fn, args = units[ui]
                                        fn(*args)
                                        ui += 1
                            emit_norm_half(TD - 1, 0)
                            while ui < len(units):
                                fn, args = units[ui]
                                fn(*args)
                                ui += 1
                        else:
                            for h in range(HG):
                                if h in (1, 2, 4):
                                    t_ = {1: 1, 2: 2, 4: 3}[h]
                                    proj_qk_t(xq1, ws3["q"], biasq, qt,
                                              "q", 1, 0, t_)
                                    proj_qk_t(xk1, ws3["k"], biask, kt,
                                              "k", 1, 0, t_)
                                emit_head_half(h, 1)
                                if h % 2 == 1 and h >= 3:
                                    emit_norm_half((h - 3) // 2, 1)
                                # D's q-half-0 only needs the half-0 norms,
                                # which all completed in half-0 attention:
                                # fill half-1's PE idle with these groups
                                emit_d(h, 0, act_ok=False)
                            # first two output-projection groups' t<3
                            # accumulation depends only on xtu[0..2], so PE
                            # works through it while the pair-3 Z chain
                            # (ztmp copy -> recip -> zb) completes
                            pre = []
                            for e in range(2):
                                pe_t = ps.tile(
                                    [P, 1024], F32,
                                    tag="pp" if e % 2 == 0 else "sc",
                                    name=f"pe{e}_1")
                                for t in range(TD - 1):
                                    for cs in range(2):
                                        c0 = 1024 + cs * BANK
                                        nc.tensor.matmul(
                                            pe_t[:, cs * BANK:(cs + 1) * BANK],
                                            wo_s[t][:, e * P:(e + 1) * P],
                                            xtu[t][:, c0:c0 + BANK],
                                            start=(t == 0), stop=False,
                                        )
                                pre.append(pe_t)
                            emit_norm_half(TD - 1, 1)
                            for e in range(2):
                                pe_t = pre[e]
                                for cs in range(2):
                                    c0 = 1024 + cs * BANK
                                    nc.tensor.matmul(
                                        pe_t[:, cs * BANK:(cs + 1) * BANK],
                                        wo_s[TD - 1][:, e * P:(e + 1) * P],
                                        xtu[TD - 1][:, c0:c0 + BANK],
                                        start=False, stop=True,
                                    )
                                ot = outp.tile([P, 1024], F32, tag="ot",
                                               bufs=4, name=f"otp{e}_1")
                                if e % 2 == 1:
                                    nc.scalar.copy(ot[:], pe_t[:])
                                else:
                                    nc.vector.tensor_copy(ot[:], pe_t[:])
                                nc.sync.dma_start(y_t[e][:, 1024:2048], ot[:])
                            for e in range(2, KD):
                                emit_d(e, 1, act_ok=True)
                else:
                    xs = load_x8(xq_t, "q", None)
                    ws3["q"] = load_w8(wq_t, "q")
                    proj_qk(xs, ws3["q"], biasq, qt, "q", 0, 0)
                    proj_qk(xs, ws3["q"], biasq, qt, "q", 1, 1024)
                    xs = load_x8(xk_t, "k", None)
                    ws3["k"] = load_w8(wk_t, "k")
                    proj_qk(xs, ws3["k"], biask, kt, "k", 0, 0)
                    proj_qk(xs, ws3["k"], biask, kt, "k", 1, 1024)
                    xs = load_x8_bf(xv_t, "v", None)
                    ws3["v"] = []
                    for i in range(KD):
                        wt_ = win.tile([P, DG], BF16, tag="w", bufs=win_b,
                                       name=f"wvn{i}")
                        nc.sync.dma_start(wt_[:], wv_t[i])
                        ws3["v"].append(wt_)
                    proj_v(xs, range(NT), 0)
                    late_loads()
                    for h in range(HG):
                        for half in range(2):
                            emit_head_half(h, half)
                            if h % 2 == 1 and h >= 3:
                                emit_norm_half((h - 3) // 2, half)
                    emit_norm_half(TD - 1, 0)
                    emit_norm_half(TD - 1, 1)
                    for e in range(KD):
                        for half in range(2):
                            emit_d(e, half, act_ok=True)

    nc.compile()
    return nc


def _get_nc(mask_bool, has_bias):
    key = (hash(mask_bool.tobytes()), has_bias)
    if key not in _cache:
        plan = _plan_from_mask(mask_bool, has_bias)
        _cache[key] = (_build(plan), plan)
    return _cache[key]


def kernel(query, key, value, mask, Wq, bq, Wk, bk, Wv, bv, Wo, bo):
    global _last_results
    bf = ml_dtypes.bfloat16
    f8 = ml_dtypes.float8_e4m3
    query = np.asarray(query, dtype=np.float32)
    key = np.asarray(key, dtype=np.float32)
    value = np.asarray(value, dtype=np.float32)
    Wq = np.asarray(Wq, dtype=np.float32)
    Wk = np.asarray(Wk, dtype=np.float32)
    Wv = np.asarray(Wv, dtype=np.float32)
    Wo = np.asarray(Wo, dtype=np.float32)
    bq = np.asarray(bq, dtype=np.float32)
    bk = np.asarray(bk, dtype=np.float32)
    bv = np.asarray(bv, dtype=np.float32)
    bo = np.asarray(bo, dtype=np.float32)
    mask_bool = np.asarray(mask).reshape(S, S) != 0
    has_bias = bool(np.any(bq) or np.any(bk) or np.any(bv))

    nc, plan = _get_nc(mask_bool, has_bias)

    # head-parity selectors for the 1/Z broadcast matmuls: row 64 (the
    # partition the Z rows live on), column block hh selects the d-columns
    # of head-parity hh within a pair's 128-row dtile
    ind = np.zeros((HC, 2 * P), np.float32)
    for hh in range(2):
        for m in range(P):
            if m // DH == hh:
                ind[DH, hh * P + m] = 1.0

    nmix = max(1, len(plan["mixed_tiles"]))
    mm = np.zeros((nmix, P, P), bf)
    for idx, (i, j) in enumerate(plan["mixed_tiles"]):
        mm[idx] = mask_bool[i * P:(i + 1) * P, j * P:(j + 1) * P].T.astype(bf)

    in_maps = []
    xq_f8 = [query[b].T.astype(f8) for b in range(B)]
    xk_f8 = [key[b].T.astype(f8) for b in range(B)]
    xv_bf = [value[b].T.astype(bf) for b in range(B)]
    for c in range(8):
        b, g = c // 2, c % 2
        gc = slice(g * DG, (g + 1) * DG)
        im = {
            "xq": xq_f8[b],
            "xk": xk_f8[b],
            "xv": xv_bf[b],
            "wq": (Wq[:, gc] * WS).astype(f8),
            "wk": (Wk[:, gc] * WS).astype(f8),
            "wv": Wv[:, gc].astype(bf),
            "wo": Wo[gc, :].astype(bf),
            "ind": ind,
            "mmask": mm,
        }
        if has_bias:
            im["bq"] = (bq[gc] * WS).reshape(1, DG).astype(bf)
            im["bk"] = (bk[gc] * WS).reshape(1, DG).astype(bf)
            im["bv"] = bv[gc].reshape(1, DG).astype(bf)
        in_maps.append(im)

    global _last_in_maps
    _last_in_maps = in_maps
    res = bass_utils.run_bass_kernel_spmd(nc, in_maps, core_ids=list(range(8)))
    _last_results = res

    out = np.empty((B, S, D), np.float32)
    for b in range(B):
        yT = res.results[2 * b]["yT"] + res.results[2 * b + 1]["yT"]
        out[b] = yT.T + bo
    return out



# revision 37
# speedup vs baseline: 1.1506x; 1.0238x over previous
"""Multi-head attention (B=4, S=2048, D=1024, H=16, DH=64) on 8 Trainium2
NeuronCores.

Sharding: core c handles batch b = c//2 and head-group g = c%2 (8 heads,
i.e. columns 512g:512(g+1) of Wq/Wk/Wv and rows 512g:512(g+1) of Wo).
Each core produces a partial output projection; the host sums the two
partials per batch and adds bo. No collectives.

Device kernel (per core, everything bf16 with fp32 PSUM accumulation):
  A. QT = Wq_g^T @ xq^T   [512, 2048]   (likewise KT), V = xv @ Wv_g
     stored interleaved with a ones column per head ("vext").
  B. Per head h, per key-tile j: scoresT[k, q] = KT_j^T-stationary matmul,
     exp via ScalarE (scale=1/sqrt(DH) folded in, no max subtraction -- the
     scores are bounded), causal/masked tiles handled by host-computed tile
     classification (skip / elementwise-multiply).  PV matmul with
     lhsT = [V | ones] accumulates unnormalized outT plus the softmax
     denominators Z in one pass.
  C. Normalize: xT *= broadcast(1/Z) (indicator-matrix matmul broadcast).
  D. yT = Wo_g-stationary projection of xT, written transposed; host
     re-transposes.
"""

import numpy as np
import ml_dtypes

import concourse.bacc as bacc
import concourse.mybir as mybir
import concourse.tile as tile
from concourse import bass_utils

BF16 = mybir.dt.bfloat16
F32 = mybir.dt.float32
F32R = mybir.dt.float32r
FP8 = mybir.dt.float8e4
DR = mybir.MatmulPerfMode.DoubleRow
EXP = mybir.ActivationFunctionType.Exp

B, S, D, H, DH = 4, 2048, 1024, 16, 64
P = 128
NT = S // P            # 16 key/query tiles
GROUPS = 2             # head groups (tensor parallel)
HG = H // GROUPS       # 8 heads per core
DG = D // GROUPS       # 512
KD = D // P            # 8 contraction tiles over D
KP = KD // 2           # 4 DoubleRow contraction pairs over D
TD = DG // P           # 4 d-tiles per group
HC = DH + 1            # 65: V columns + ones column per head
# fp8 weight scale: W*0.02 is subnormal in e4m3; scale by 16 so weights
# land in the normal range.  q/k come out 16x too big (exp scale folds
# 1/256), V comes out 16x too big (the vext "ones" column is 16 so the
# computed 1/Z absorbs it exactly -- powers of two, no rounding).
WS = 16.0
SCALE = float(DH) ** -0.5 / (WS * WS)
BANK = 512             # fp32 PSUM bank, in elements
MAX_PRELOAD_MASK = 64
# folded q/k layout for fp8 DoubleRow scores: 3 heads per [128, 2, S]
# tile, head h -> tile h//3, partition quadrant 32*(h%3)+c, dh = 32*i+c
# (quadrant base 96 is not addressable, hence 3 heads/tile).  The q/k
# projections write 6 permuted 128-column blocks (tile, fold) each.
FB = 6                 # folded projection column blocks
DGF = FB * P           # 768: permuted q/k weight width (incl. pad)

_cache = {}
_last_results = None


def _plan_from_mask(mask_bool, has_bias):
    g = mask_bool.reshape(NT, P, NT, P).sum(axis=(1, 3))
    full = g == P * P
    zero = g == 0

    mixed_tiles = []        # ordered list of (i, j)
    mixed_of = {}           # (i, j) -> index into mixed_tiles
    j_info = [None] * NT
    for j in range(NT):
        act = [i for i in range(NT) if not zero[i, j]]
        if not act:
            continue
        i0, i1 = min(act), max(act) + 1
        mixed = []
        for i in range(i0, i1):
            if not full[i, j]:
                if (i, j) not in mixed_of:
                    mixed_of[(i, j)] = len(mixed_tiles)
                    mixed_tiles.append((i, j))
                mixed.append((i, mixed_of[(i, j)]))
        qlo, qhi = i0 * P, i1 * P
        # one slab per 1024-wide q-half; slab PSUM tile base s0a is
        # 512-aligned so the 512-aligned matmul chunks never cross a bank
        # inside the tile.
        slabs = {}
        for half in range(2):
            qb = max(qlo, half * 1024)
            qe = min(qhi, (half + 1) * 1024)
            if qb >= qe:
                continue
            s0a = (qb // BANK) * BANK
            chunks = []
            d = qb
            while d < qe:
                d2 = min((d // BANK + 1) * BANK, qe)
                chunks.append((d, d2))
                d = d2
            slabs[half] = (s0a, qb, qe, chunks)
        j_info[j] = dict(qlo=qlo, qhi=qhi, slabs=slabs, mixed=mixed)

    first_j = {}
    last_j = {}
    for j in range(NT):
        if j_info[j] is None:
            continue
        for (_, _, _, chunks) in j_info[j]["slabs"].values():
            for (c0, _) in chunks:
                bk = c0 // BANK
                first_j.setdefault(bk, j)
                last_j[bk] = j
    # split-schedule legality: q-half-0 attention touches only key-half-0
    # (true for causal), so projections can be computed half-by-half with
    # attention interleaved between them
    split = all(j_info[j] is None or 0 not in j_info[j]["slabs"]
                for j in range(NT // 2, NT))
    return dict(
        j_info=j_info,
        mixed_tiles=mixed_tiles,
        first_j=first_j,
        last_j=last_j,
        has_bias=has_bias,
        split=split,
    )


def _build(plan):
    has_bias = plan["has_bias"]
    j_info = plan["j_info"]
    mixed_tiles = plan["mixed_tiles"]
    nm = max(1, len(mixed_tiles))
    preload = len(mixed_tiles) <= MAX_PRELOAD_MASK

    nc = bacc.Bacc("TRN2", target_bir_lowering=False, debug=False)
    xq_d = nc.dram_tensor("xq", [D, S], FP8, kind="ExternalInput").ap()
    xk_d = nc.dram_tensor("xk", [D, S], FP8, kind="ExternalInput").ap()
    xv_d = nc.dram_tensor("xv", [D, S], BF16, kind="ExternalInput").ap()
    wq_d = nc.dram_tensor("wq", [D, DGF], FP8, kind="ExternalInput").ap()
    wk_d = nc.dram_tensor("wk", [D, DGF], FP8, kind="ExternalInput").ap()
    wv_d = nc.dram_tensor("wv", [D, DG], BF16, kind="ExternalInput").ap()
    wo_d = nc.dram_tensor("wo", [DG, D], BF16, kind="ExternalInput").ap()
    ind_d = nc.dram_tensor("ind", [HC, 2 * P], F32R, kind="ExternalInput").ap()
    mm_d = nc.dram_tensor("mmask", [nm, P, P], BF16, kind="ExternalInput").ap()
    if has_bias:
        bq_d = nc.dram_tensor("bq", [1, DGF], BF16, kind="ExternalInput").ap()
        bk_d = nc.dram_tensor("bk", [1, DGF], BF16, kind="ExternalInput").ap()
        bv_d = nc.dram_tensor("bv", [1, DG], BF16, kind="ExternalInput").ap()
    y_d = nc.dram_tensor("yT", [D, S], F32, kind="ExternalOutput").ap()

    # DoubleRow pair views: contraction row (2i*128+s*128+p) -> [i][p, s, .]
    xq_t = xq_d.rearrange("(n s p) q -> n p s q", p=P, s=2)
    xk_t = xk_d.rearrange("(n s p) q -> n p s q", p=P, s=2)
    xv_t = xv_d.rearrange("(n p) q -> n p q", p=P)
    wq_t = wq_d.rearrange("(n s p) d -> n p s d", p=P, s=2)
    wk_t = wk_d.rearrange("(n s p) d -> n p s d", p=P, s=2)
    wv_t = wv_d.rearrange("(n p) d -> n p d", p=P)
    wo_t = wo_d.rearrange("(n p) e -> n p e", p=P)
    y_t = y_d.rearrange("(n p) q -> n p q", p=P)

    with tile.TileContext(nc, trace_sim=False) as tc:
        with (
            tc.tile_pool(name="pers", bufs=1) as pers,
            tc.tile_pool(name="xin", bufs=12) as xin,
            tc.tile_pool(name="win", bufs=10) as win,
            tc.tile_pool(name="ptp", bufs=4) as ptp,
            tc.tile_pool(name="tmpp", bufs=1) as tmpp,
            tc.tile_pool(name="outp", bufs=2) as outp,
        ):
            # ---- persistent SBUF tensors -------------------------------
            qf = [pers.tile([P, 2, S], FP8, tag="qf", bufs=3, name=f"qf{T}")
                  for T in range(3)]
            kf = [pers.tile([P, 2, S], FP8, tag="kf", bufs=3, name=f"kf{T}")
                  for T in range(3)]
            vx = [pers.tile([P, HG * HC], BF16, tag="vx", bufs=NT,
                            name=f"vx{j}") for j in range(NT)]
            xtu = [pers.tile([P, S], BF16, tag="xtu", bufs=TD, name=f"xtu{t}")
                   for t in range(TD)]
            # head-parity selector rows for the 1/Z broadcast matmuls; row 64
            # so the base partition matches the ztmp Z-rows (bass requires
            # equal lhsT/rhs base partitions)
            ind_s = pers.tile([HC, 2 * P], F32R, tag="ind", bufs=1,
                              name="ind_s")
            wo_s = [pers.tile([P, D], BF16, tag="wo", bufs=TD, name=f"wo{t}")
                    for t in range(TD)]

            mtile = {}
            if preload:
                for idx, (i, j) in enumerate(mixed_tiles):
                    mtile[(i, j)] = pers.tile([P, P], BF16, tag="mt", bufs=nm,
                                              name=f"mt{idx}")

            if has_bias:
                ones = pers.tile([1, BANK], BF16, tag="ones", bufs=1,
                                 name="ones")
                nc.vector.memset(ones[:], 1.0)
                bias_s = {}
                for nm_, d_, w_ in (("bq", bq_d, DGF), ("bk", bk_d, DGF),
                                    ("bv", bv_d, DG)):
                    bs = pers.tile([1, w_], BF16, tag="bias", bufs=3,
                                   name=f"{nm_}_s")
                    nc.sync.dma_start(bs[:], d_)
                    bias_s[nm_] = bs

            # ---- Phases A+B share one PSUM pool ------------------------
            #   tag "pp" (2x2 banks): projection psums (A), pout tiles (B)
            #   tag "sc" (2x2 banks): V-proj psums, score slabs, zb tiles
            # Sharing tags across phases keeps the slots flowing with no
            # pool-boundary barrier, so V-proj overlaps early attention.
            with tc.tile_pool(name="psAB", bufs=2, space="PSUM") as ps:
                split = plan["split"]
                xin_b = 22 if split else 12
                win_b = 20 if split else 16
                ztmp_b = 6 if split else 8

                def load_w8(wd, label):
                    ts_ = []
                    for i in range(KP):
                        wt_ = win.tile([P, 2, DGF], FP8, tag="w", bufs=win_b,
                                       name=f"w{label}{i}")
                        nc.sync.dma_start(wt_[:], wd[i])
                        ts_.append(wt_)
                    return ts_

                def load_x8(xd, label, half):
                    # half=None: full rows (serial); else one 1024-col half
                    w = S if half is None else 1024
                    off = 0 if half is None else 1024 * half
                    ts_ = []
                    for i in range(KP):
                        xt_ = xin.tile([P, 2, w], FP8, tag="x", bufs=xin_b,
                                       name=f"x{label}{i}")
                        nc.sync.dma_start(xt_[:], xd[i][:, :, off:off + w])
                        ts_.append(xt_)
                    return ts_

                def load_x8_bf(xd, label, half):
                    w = S if half is None else 1024
                    off = 0 if half is None else 1024 * half
                    ts_ = []
                    for i in range(KD):
                        xt_ = xin.tile([P, w], BF16, tag="x", bufs=xin_b,
                                       name=f"x{label}{i}")
                        nc.sync.dma_start(xt_[:], xd[i][:, off:off + w])
                        ts_.append(xt_)
                    return ts_

                def proj_qk_t(xs, ws, bias, out_tiles, label, half, xoff,
                              blk):
                    # block blk = (tile T=blk//2, fold i=blk%2): psum
                    # partition 32a+c = head 3T+a, dh 32i+c (host-permuted W)
                    T, fi = blk // 2, blk % 2
                    pp = ps.tile([P, 1024], F32, tag="pp",
                                 name=f"ps_{label}{blk}_{half}")
                    for i in range(KP):
                        for cs in range(2):
                            x0 = xoff + cs * BANK
                            nc.tensor.matmul(
                                pp[:, cs * BANK:(cs + 1) * BANK],
                                ws[i][:, :, blk * P:(blk + 1) * P],
                                xs[i][:, :, x0:x0 + BANK],
                                start=(i == 0),
                                stop=(i == KP - 1 and bias is None),
                                perf_mode=DR,
                            )
                    if bias is not None:
                        for cs in range(2):
                            nc.tensor.matmul(
                                pp[:, cs * BANK:(cs + 1) * BANK],
                                bias[0:1, blk * P:(blk + 1) * P],
                                ones[0:1, :],
                                start=False, stop=(cs == 1),
                            )
                    with nc.allow_low_precision(reason="fp8 folded q/k"):
                        nc.vector.tensor_copy(
                            out_tiles[T][:, fi,
                                         half * 1024:(half + 1) * 1024],
                            pp[:],
                        )

                def proj_qk(xs, ws, bias, out_tiles, label, half, xoff):
                    for blk in range(FB):
                        proj_qk_t(xs, ws, bias, out_tiles, label, half,
                                  xoff, blk)

                def proj_v(xs, jrange, xoff_base):
                    for j in jrange:
                        lc = j * P - xoff_base
                        psv = ps.tile([P, DG], F32, tag="pp", name=f"ps_v{j}")
                        for i in range(KD):
                            nc.tensor.matmul(
                                psv[:],
                                xs[i][:, lc:lc + P],
                                ws3["v"][i][:],
                                start=(i == 0),
                                stop=(i == KD - 1 and not has_bias),
                            )
                        if has_bias:
                            nc.tensor.matmul(
                                psv[:], ones[0:1, 0:P], bias_s["bv"][0:1, :],
                                start=False, stop=True,
                            )
                        vxv = vx[j][:].rearrange("p (g c) -> p g c", c=HC)
                        nc.vector.memset(vxv[:, :, DH:HC], 1.0)
                        nc.vector.tensor_copy(
                            vxv[:, :, 0:DH],
                            psv[:].rearrange("p (g c) -> p g c", c=DH),
                        )

                def late_loads():
                    # needed only from phase B onward; emitted after the x/w
                    # loads so they queue behind them on the DMA engines
                    nc.sync.dma_start(ind_s[:], ind_d)
                    for t in range(TD):
                        nc.sync.dma_start(wo_s[t][:], wo_t[t])
                    if preload:
                        for idx, (i, j) in enumerate(mixed_tiles):
                            nc.sync.dma_start(mtile[(i, j)][:], mm_d[idx])

                ztmps = {}

                def emit_norm_half(t, half):
                    # normalize xtu[t] q-half by 1/Z of head pair (2t, 2t+1)
                    zb = ps.tile([P, 1024], F32, tag="pp",
                                 name=f"zb{t}_{half}")
                    for hh in range(2):
                        zt_ = ztmps[(2 * t + hh, half)]
                        for cs in range(2):
                            nc.tensor.matmul(
                                zb[:, cs * BANK:(cs + 1) * BANK],
                                ind_s[DH:HC, hh * P:(hh + 1) * P],
                                zt_[DH:HC, cs * BANK:(cs + 1) * BANK],
                                start=(hh == 0), stop=(hh == 1),
                            )
                    nc.vector.tensor_mul(
                        xtu[t][:, half * 1024:(half + 1) * 1024],
                        xtu[t][:, half * 1024:(half + 1) * 1024],
                        zb[:],
                    )

                def emit_head_half(h, half, mid=None):
                    # `mid` = filler work (projection units, deferred norms,
                    # output-projection groups) emitted after the 4th key
                    # tile: mid-head DVE is idle, so the fillers' PSUM slots
                    # release promptly instead of queueing behind the
                    # head-boundary copy burst and starving ScalarE
                    t, r0 = h // 2, DH * (h % 2)
                    h0, h1 = half * 1024, (half + 1) * 1024
                    pout_t = ps.tile([P, 1024], F32, tag="pp",
                                     name=f"pout{h}_{half}")
                    pout = pout_t[0:HC]
                    wrote = False
                    for j in range(NT):
                        info = j_info[j]
                        if info is None or half not in info["slabs"]:
                            continue
                        (s0, qb, s1, chunks) = info["slabs"][half]
                        ps_s = ps.tile([P, 1024], F32, tag="sc",
                                       name=f"sc{h}_{j}_{half}")
                        Tq, aq = h // 3, 32 * (h % 3)
                        for (c0, c1) in chunks:
                            nc.tensor.matmul(
                                ps_s[:, c0 - s0:c1 - s0],
                                kf[Tq][aq:aq + 32, :, j * P:(j + 1) * P],
                                qf[Tq][aq:aq + 32, :, c0:c1],
                                start=True, stop=True, perf_mode=DR,
                            )
                        pt = ptp.tile([P, 1024], BF16, tag="pt", bufs=4,
                                      name=f"pt{h}_{j}_{half}")
                        nc.scalar.activation(
                            pt[:, qb - s0:s1 - s0],
                            ps_s[:, qb - s0:s1 - s0], EXP,
                            scale=SCALE,
                        )
                        for (i, idx) in info["mixed"]:
                            ic = i * P
                            if not (qb <= ic < s1):
                                continue
                            if preload:
                                mt = mtile[(i, j)]
                            else:
                                mt = ptp.tile([P, P], BF16, tag="mts",
                                              bufs=4, name=f"mts{h}_{j}_{i}")
                                nc.sync.dma_start(mt[:], mm_d[idx])
                            nc.gpsimd.tensor_mul(
                                pt[:, ic - s0:ic - s0 + P],
                                pt[:, ic - s0:ic - s0 + P],
                                mt[:],
                            )
                        for (c0, c1) in chunks:
                            bk_ = c0 // BANK
                            nc.tensor.matmul(
                                pout[:, c0 - h0:c1 - h0],
                                vx[j][:, h * HC:(h + 1) * HC],
                                pt[:, c0 - s0:c1 - s0],
                                start=(j == plan["first_j"][bk_]),
                                stop=(j == plan["last_j"][bk_]),
                            )
                        wrote = True
                    if mid:
                        for fn_, args_ in mid:
                            fn_(*args_)
                    if not wrote:
                        return
                    # copy unnormalized head output + denominators out
                    if r0 == 0:
                        nc.vector.tensor_copy(xtu[t][0:DH, h0:h1],
                                              pout[0:DH, :])
                    else:
                        xtmp = tmpp.tile([DH, 1024], BF16, tag="xtmp",
                                         bufs=2, name=f"xtmp{h}_{half}")
                        nc.vector.tensor_copy(xtmp[:], pout[0:DH, :])
                        nc.sync.dma_start(xtu[t][DH:P, h0:h1], xtmp[:])
                    ztmp = tmpp.tile([HC, 1024], F32R, tag="ztmp",
                                     bufs=ztmp_b, name=f"ztmp{h}_{half}")
                    with nc.allow_low_precision(
                            reason="1/Z broadcast via f32r matmul"):
                        nc.vector.reciprocal(ztmp[DH:HC, :], pout[DH:HC, :])
                    ztmps[(h, half)] = ztmp

                def emit_d(e, half, act_ok):
                    # output projection yT[e-tile, q-half], transposed
                    g = e * 2 + half
                    pe_t = ps.tile([P, 1024], F32,
                                   tag="pp" if g % 2 == 0 else "sc",
                                   name=f"pe{e}_{half}")
                    for t in range(TD):
                        for cs in range(2):
                            c0 = half * 1024 + cs * BANK
                            nc.tensor.matmul(
                                pe_t[:, cs * BANK:(cs + 1) * BANK],
                                wo_s[t][:, e * P:(e + 1) * P],
                                xtu[t][:, c0:c0 + BANK],
                                start=(t == 0), stop=(t == TD - 1),
                            )
                    ot = outp.tile([P, 1024], F32, tag="ot", bufs=4,
                                   name=f"ot{e}_{half}")
                    if act_ok and g % 2 == 1:
                        nc.scalar.copy(ot[:], pe_t[:])
                    else:
                        nc.vector.tensor_copy(ot[:], pe_t[:])
                    nc.sync.dma_start(
                        y_t[e][:, half * 1024:(half + 1) * 1024], ot[:])

                biasq = bias_s["bq"] if has_bias else None
                biask = bias_s["bk"] if has_bias else None
                ws3 = {}
                if split:
                    # causal-style masks: q-half-0 attention uses only
                    # key-half-0, so project half-by-half with attention
                    # interleaved -- ScalarE exp hides the projections
                    for half in range(2):
                        if half == 0:
                            # interleave w and x DMAs so the first matmul's
                            # operands land early in the queues
                            ws3["q"], ws3["k"], ws3["v"] = [], [], []
                            xs = []
                            for i in range(KP):
                                ws3["q"].append(win.tile(
                                    [P, 2, DGF], FP8, tag="w", bufs=win_b,
                                    name=f"wq{i}"))
                                nc.sync.dma_start(ws3["q"][i][:], wq_t[i])
                                xt_ = xin.tile([P, 2, 1024], FP8, tag="x",
                                               bufs=xin_b, name=f"xq0_{i}")
                                nc.sync.dma_start(xt_[:], xq_t[i][:, :, 0:1024])
                                xs.append(xt_)
                            xk0, xv0 = [], []
                            for i in range(KP):
                                ws3["k"].append(win.tile(
                                    [P, 2, DGF], FP8, tag="w", bufs=win_b,
                                    name=f"wk{i}"))
                                nc.sync.dma_start(ws3["k"][i][:], wk_t[i])
                                xt_ = xin.tile([P, 2, 1024], FP8, tag="x",
                                               bufs=xin_b, name=f"xk0_{i}")
                                nc.sync.dma_start(xt_[:], xk_t[i][:, :, 0:1024])
                                xk0.append(xt_)
                            for i in range(KD):
                                ws3["v"].append(win.tile(
                                    [P, DG], BF16, tag="w", bufs=win_b,
                                    name=f"wv{i}"))
                                nc.sync.dma_start(ws3["v"][i][:], wv_t[i])
                                xt_ = xin.tile([P, 1024], BF16, tag="x",
                                               bufs=xin_b, name=f"xv0_{i}")
                                nc.sync.dma_start(xt_[:], xv_t[i][:, 0:1024])
                                xv0.append(xt_)
                            proj_qk(xs, ws3["q"], biasq, qf, "q", 0, 0)
                            proj_qk(xk0, ws3["k"], biask, kf, "k", 0, 0)
                            proj_v(xv0, range(8), 0)
                            late_loads()
                            # half-1 projection work interleaved into half-0
                            # attention (ScalarE-bound): V and dtiles 0-1
                            # here; dtiles 2-3 go into half-1 attention,
                            # which is also ScalarE-bound
                            units = []
                            xv1 = load_x8_bf(xv_t, "v1", 1)
                            for j_ in range(8, NT):
                                units.append((proj_v, (xv1, [j_], 1024)))
                            xq1 = load_x8(xq_t, "q1", 1)
                            xk1 = load_x8(xk_t, "k1", 1)
                            units.append((proj_qk_t, (xq1, ws3["q"],
                                          biasq, qf, "q", 1, 0, 0)))
                            units.append((proj_qk_t, (xk1, ws3["k"],
                                          biask, kf, "k", 1, 0, 0)))
                            units.append((proj_qk_t, (xq1, ws3["q"],
                                          biasq, qf, "q", 1, 0, 1)))
                            units.append((proj_qk_t, (xk1, ws3["k"],
                                          biask, kf, "k", 1, 0, 1)))
                            ui = 0
                            for h in range(HG):
                                emit_head_half(h, 0)
                                if h % 2 == 1 and h >= 3:
                                    emit_norm_half((h - 3) // 2, 0)
                                for _ in range(2 if h < 4 else 1):
                                    if ui < len(units):
                                        fn, args = units[ui]
                                        fn(*args)
                                        ui += 1
                            emit_norm_half(TD - 1, 0)
                            while ui < len(units):
                                fn, args = units[ui]
                                fn(*args)
                                ui += 1
                        else:
                            for h in range(HG):
                                if h in (1, 2, 4, 5):
                                    blk_ = {1: 2, 2: 3, 4: 4, 5: 5}[h]
                                    proj_qk_t(xq1, ws3["q"], biasq, qf,
                                              "q", 1, 0, blk_)
                                    proj_qk_t(xk1, ws3["k"], biask, kf,
                                              "k", 1, 0, blk_)
                                emit_head_half(h, 1)
                                if h % 2 == 1 and h >= 3:
                                    emit_norm_half((h - 3) // 2, 1)
                                # D's q-half-0 only needs the half-0 norms,
                                # which all completed in half-0 attention:
                                # fill half-1's PE idle with these groups
                                emit_d(h, 0, act_ok=False)
                            # first two output-projection groups' t<3
                            # accumulation depends only on xtu[0..2], so PE
                            # works through it while the pair-3 Z chain
                            # (ztmp copy -> recip -> zb) completes
                            pre = []
                            for e in range(2):
                                pe_t = ps.tile(
                                    [P, 1024], F32,
                                    tag="pp" if e % 2 == 0 else "sc",
                                    name=f"pe{e}_1")
                                for t in range(TD - 1):
                                    for cs in range(2):
                                        c0 = 1024 + cs * BANK
                                        nc.tensor.matmul(
                                            pe_t[:, cs * BANK:(cs + 1) * BANK],
                                            wo_s[t][:, e * P:(e + 1) * P],
                                            xtu[t][:, c0:c0 + BANK],
                                            start=(t == 0), stop=False,
                                        )
                                pre.append(pe_t)
                            emit_norm_half(TD - 1, 1)
                            for e in range(2):
                                pe_t = pre[e]
                                for cs in range(2):
                                    c0 = 1024 + cs * BANK
                                    nc.tensor.matmul(
                                        pe_t[:, cs * BANK:(cs + 1) * BANK],
                                        wo_s[TD - 1][:, e * P:(e + 1) * P],
                                        xtu[TD - 1][:, c0:c0 + BANK],
                                        start=False, stop=True,
                                    )
                                ot = outp.tile([P, 1024], F32, tag="ot",
                                               bufs=4, name=f"otp{e}_1")
                                if e % 2 == 1:
                                    nc.scalar.copy(ot[:], pe_t[:])
                                else:
                                    nc.vector.tensor_copy(ot[:], pe_t[:])
                                nc.sync.dma_start(y_t[e][:, 1024:2048], ot[:])
                            for e in range(2, KD):
                                emit_d(e, 1, act_ok=True)
                else:
                    xs = load_x8(xq_t, "q", None)
                    ws3["q"] = load_w8(wq_t, "q")
                    proj_qk(xs, ws3["q"], biasq, qf, "q", 0, 0)
                    proj_qk(xs, ws3["q"], biasq, qf, "q", 1, 1024)
                    xs = load_x8(xk_t, "k", None)
                    ws3["k"] = load_w8(wk_t, "k")
                    proj_qk(xs, ws3["k"], biask, kf, "k", 0, 0)
                    proj_qk(xs, ws3["k"], biask, kf, "k", 1, 1024)
                    xs = load_x8_bf(xv_t, "v", None)
                    ws3["v"] = []
                    for i in range(KD):
                        wt_ = win.tile([P, DG], BF16, tag="w", bufs=win_b,
                                       name=f"wvn{i}")
                        nc.sync.dma_start(wt_[:], wv_t[i])
                        ws3["v"].append(wt_)
                    proj_v(xs, range(NT), 0)
                    late_loads()
                    for h in range(HG):
                        for half in range(2):
                            emit_head_half(h, half)
                            if h % 2 == 1 and h >= 3:
                                emit_norm_half((h - 3) // 2, half)
                    emit_norm_half(TD - 1, 0)
                    emit_norm_half(TD - 1, 1)
                    for e in range(KD):
                        for half in range(2):
                            emit_d(e, half, act_ok=True)

    nc.compile()
    return nc


def _get_nc(mask_bool, has_bias):
    key = (hash(mask_bool.tobytes()), has_bias)
    if key not in _cache:
        plan = _plan_from_mask(mask_bool, has_bias)
        _cache[key] = (_build(plan), plan)
    return _cache[key]


def kernel(query, key, value, mask, Wq, bq, Wk, bk, Wv, bv, Wo, bo):
    global _last_results
    bf = ml_dtypes.bfloat16
    f8 = ml_dtypes.float8_e4m3
    query = np.asarray(query, dtype=np.float32)
    key = np.asarray(key, dtype=np.float32)
    value = np.asarray(value, dtype=np.float32)
    Wq = np.asarray(Wq, dtype=np.float32)
    Wk = np.asarray(Wk, dtype=np.float32)
    Wv = np.asarray(Wv, dtype=np.float32)
    Wo = np.asarray(Wo, dtype=np.float32)
    bq = np.asarray(bq, dtype=np.float32)
    bk = np.asarray(bk, dtype=np.float32)
    bv = np.asarray(bv, dtype=np.float32)
    bo = np.asarray(bo, dtype=np.float32)
    mask_bool = np.asarray(mask).reshape(S, S) != 0
    has_bias = bool(np.any(bq) or np.any(bk) or np.any(bv))

    nc, plan = _get_nc(mask_bool, has_bias)

    # head-parity selectors for the 1/Z broadcast matmuls: row 64 (the
    # partition the Z rows live on), column block hh selects the d-columns
    # of head-parity hh within a pair's 128-row dtile
    ind = np.zeros((HC, 2 * P), np.float32)
    for hh in range(2):
        for m in range(P):
            if m // DH == hh:
                ind[DH, hh * P + m] = 1.0

    nmix = max(1, len(plan["mixed_tiles"]))
    mm = np.zeros((nmix, P, P), bf)
    for idx, (i, j) in enumerate(plan["mixed_tiles"]):
        mm[idx] = mask_bool[i * P:(i + 1) * P, j * P:(j + 1) * P].T.astype(bf)

    # column permutation for the folded q/k projections: block
    # blk=(T,i), partition 32a+c  ->  dg column 64*(3T+a) + 32*i + c
    perm = np.zeros(DGF, np.int64)
    for blk in range(FB):
        T, fi = blk // 2, blk % 2
        for p_ in range(P):
            a_, c_ = p_ // 32, p_ % 32
            hh = 3 * T + a_
            perm[blk * P + p_] = (64 * hh + 32 * fi + c_) if hh < HG else 0

    in_maps = []
    xq_f8 = [query[b].T.astype(f8) for b in range(B)]
    xk_f8 = [key[b].T.astype(f8) for b in range(B)]
    xv_bf = [value[b].T.astype(bf) for b in range(B)]
    for c in range(8):
        b, g = c // 2, c % 2
        gc = slice(g * DG, (g + 1) * DG)
        im = {
            "xq": xq_f8[b],
            "xk": xk_f8[b],
            "xv": xv_bf[b],
            "wq": (Wq[:, gc][:, perm] * WS).astype(f8),
            "wk": (Wk[:, gc][:, perm] * WS).astype(f8),
            "wv": Wv[:, gc].astype(bf),
            "wo": Wo[gc, :].astype(bf),
            "ind": ind,
            "mmask": mm,
        }
        if has_bias:
            im["bq"] = (bq[gc][perm] * WS).reshape(1, DGF).astype(bf)
            im["bk"] = (bk[gc][perm] * WS).reshape(1, DGF).astype(bf)
            im["bv"] = bv[gc].reshape(1, DG).astype(bf)
        in_maps.append(im)

    global _last_in_maps
    _last_in_maps = in_maps
    res = bass_utils.run_bass_kernel_spmd(nc, in_maps, core_ids=list(range(8)))
    _last_results = res

    out = np.empty((B, S, D), np.float32)
    for b in range(B):
        yT = res.results[2 * b]["yT"] + res.results[2 * b + 1]["yT"]
        out[b] = yT.T + bo
    return out

